# revision 28
# baseline (speedup 1.0000x reference)
"""Trainium2 Bass kernel for nn_CCCrossLayerAttentionB (criss-cross cross-layer attention).

Self-contained: kernel(**inputs) -> np.ndarray [8, 512, 96, 96] fp32.

Sharding: data-parallel over batch (8 images -> 8 cores); BN stats via AllReduce.

Host-side restructuring (exact up to float assoc):
  - qf never materialized: q = (q_w @ conv1_w) @ low.
  - hf never materialized: every hf-consumer is a 1x1 conv, which commutes with the
    (separable, linear) bilinear 2x upsample, so the high stream is computed on the
    48x48 grid and upsampled afterward:
      ks = (k_w@conv2_w)@high; vs = (v_w@conv2_w)@high; xs = ((Wb_v@conv2_w+Wb_h)/16)@high
  - Device blends compute up16() = 16*up() in one fused op per output; the 1/16 is
    folded into host weights (q /16; xs /16; final att weights gamma/16).
  - vf2 never materialized:  k2' = (gamma k_w)@A1' + k1',  v2' = (gamma v_w)@A1' + v1',
    y = (gamma/16 Wb_v)@(A1'+A2') + up16(xs);  A' accumulated in place, h-major (c,h,w).
Attention per column w (and symmetrically per row h):
  e[H',h] = k'[:,H',w]^T q'[:,h,w] (flipped => softmax sums via ones-matmul, aggregation
  needs no attention transpose);  exp via ACT (+ -1e9 diag mask for the H direction);
  normalize exp tiles; aggregate with pixel-major v slices obtained from per-column
  PE transposes.
y never leaves SBUF: accumulated per-strip into resident (c,h,w) tiles with running
per-channel sum / sum-of-squares; after the stats AllReduce, BN+ReLU is applied from
SBUF and the output leaves as whole contiguous (c, h, w) blocks.
"""
import numpy as np
import ml_dtypes

import concourse.bass as bass
import concourse.bacc as bacc
import concourse.tile as tile
from concourse import mybir
from concourse.bass_utils import run_bass_kernel_spmd

F32 = mybir.dt.float32
BF16 = mybir.dt.bfloat16
AL = mybir.AluOpType
AF = mybir.ActivationFunctionType

N_CORES = 8
B, C, H, W = 8, 512, 96, 96
HS = 48
PIX = H * W
PIXS = HS * HS
CIN = 256
CI = 32
NTOT = float(B * PIX)
BN_EPS = 1e-5
NEG = -1e9

_CACHE = {}


def _f32(x):
    return np.ascontiguousarray(np.asarray(x, dtype=np.float32))


# ---------------------------------------------------------------------------
# blend helpers: up16 along last dim (48 -> 96) / middle dim
# ---------------------------------------------------------------------------

def _up_last(nc, eng, out, xin, eng2=None):
    """xin [P, n, 48] -> out [P, n, 96], out = 16 * bilinear (x4 per axis)."""
    eng2 = eng2 or eng
    eng.tensor_scalar_mul(out[:, :, 0:1], xin[:, :, 0:1], 4.0)
    eng.tensor_scalar_mul(out[:, :, 95:96], xin[:, :, 47:48], 4.0)
    eng.scalar_tensor_tensor(out[:, :, 2:95:2], xin[:, :, 1:48], 3.0, xin[:, :, 0:47],
                             AL.mult, AL.add)
    eng2.scalar_tensor_tensor(out[:, :, 1:94:2], xin[:, :, 0:47], 3.0, xin[:, :, 1:48],
                              AL.mult, AL.add)


def _up_mid(nc, eng, out, xin, eng2=None):
    """xin [P, 48, n] -> out [P, 96, n]."""
    eng2 = eng2 or eng
    eng.tensor_scalar_mul(out[:, 0:1, :], xin[:, 0:1, :], 4.0)
    eng.tensor_scalar_mul(out[:, 95:96, :], xin[:, 47:48, :], 4.0)
    eng.scalar_tensor_tensor(out[:, 2:95:2, :], xin[:, 1:48, :], 3.0, xin[:, 0:47, :],
                             AL.mult, AL.add)
    eng2.scalar_tensor_tensor(out[:, 1:94:2, :], xin[:, 0:47, :], 3.0, xin[:, 1:48, :],
                              AL.mult, AL.add)


# ---------------------------------------------------------------------------
# device kernel
# ---------------------------------------------------------------------------

def build(debug_taps=False):
    nc = bacc.Bacc("TRN2", target_bir_lowering=False, debug=False, num_devices=N_CORES)

    low_d = nc.dram_tensor("low", [C, H, W], F32, kind="ExternalInput")
    high_d = nc.dram_tensor("high", [C, HS, HS], F32, kind="ExternalInput")
    wq_d = nc.dram_tensor("wqT", [C, 128], BF16, kind="ExternalInput")
    wkvx_d = nc.dram_tensor("wkvxT", [C, CIN + C + 2 * CI], BF16, kind="ExternalInput")
    wk2_d = nc.dram_tensor("wk2T", [CIN, 2 * CI], BF16, kind="ExternalInput")
    wv2_d = nc.dram_tensor("wv2T", [CIN, CIN], BF16, kind="ExternalInput")
    wfin_d = nc.dram_tensor("wfinT", [CIN, C], BF16, kind="ExternalInput")
    bnsc_d = nc.dram_tensor("bnsc", [C], F32, kind="ExternalInput")
    bnbi_d = nc.dram_tensor("bnbi", [C], F32, kind="ExternalInput")
    out_d = nc.dram_tensor("out", [C, H, W], F32, kind="ExternalOutput")

    taps = {}
    if debug_taps:
        for nm, shp in [("q", [CI, H, W]), ("k1", [CI, H, W]), ("v1", [CIN, H, W]),
                        ("expH", [96, W, 96]), ("A1", [CIN, H, W]), ("y", [C, H, W]),
                        ("k2", [CI, H, W])]:
            taps[nm] = nc.dram_tensor("t_" + nm, shp, F32, kind="ExternalOutput")

    ident = nc.inline_tensor(np.eye(128, dtype=ml_dtypes.bfloat16), "ident")
    ones_l = nc.inline_tensor(np.ones((96, 128), dtype=ml_dtypes.bfloat16), "ones_l")
    epsv = nc.inline_tensor(np.full((128, 1), BN_EPS, np.float32), "epsv")
    dmsk = np.ones((96, 4, 96), dtype=ml_dtypes.bfloat16)
    for _p in range(96):
        dmsk[_p, :, _p] = 0
    dmsk_c = nc.inline_tensor(dmsk.reshape(96, 384), "dmskc")

    st_d = [nc.dram_tensor(f"st{g}_i", [128, 2], F32) for g in range(4)]
    stg_d = [nc.dram_tensor(f"stg{g}_i", [128, 2], F32, addr_space="Shared") for g in range(4)]

    NKV = CIN + C + 2 * CI  # 832: [vs 0:256 | xs 256:768 | ks(x2) 768:832]

    with tile.TileContext(nc) as tc, (
        tc.tile_pool(name="cst", bufs=1)) as cst, (
        tc.tile_pool(name="per", bufs=1)) as per, (
        tc.tile_pool(name="strm", bufs=3)) as strm, (
        tc.tile_pool(name="pe", bufs=2, space="PSUM")) as pe, (
        tc.tile_pool(name="ps", bufs=2, space="PSUM")) as ps, (
        tc.tile_pool(name="pa", bufs=1, space="PSUM")) as pa, (
        tc.tile_pool(name="pb", bufs=2, space="PSUM")) as pb:

        # ---------------- consts & weights ----------------
        id_t = cst.tile([128, 128], BF16, tag="id")
        nc.sync.dma_start(id_t[:], ident.ap()[:])
        ones_t = cst.tile([96, 128], BF16, tag="ones")
        nc.sync.dma_start(ones_t[:], ones_l.ap()[:])
        eps_t = cst.tile([128, 1], F32, tag="eps")
        nc.sync.dma_start(eps_t[:], epsv.ap()[:])
        dmsk_t = cst.tile([96, 4, 96], BF16, tag="dmsk")
        nc.sync.dma_start(dmsk_t[:].rearrange("p a b -> p (a b)"), dmsk_c.ap()[:])

        wq_t = [cst.tile([128, 128], BF16, tag=f"wq{k}", name=f"wq{k}") for k in range(4)]
        for k in range(4):
            nc.sync.dma_start(wq_t[k][:], wq_d.ap()[k * 128:(k + 1) * 128, :])
        wk2_t = [cst.tile([128, 2 * CI], BF16, tag=f"wk2{k}", name=f"wk2{k}") for k in range(2)]
        wv2_t = [[cst.tile([128, 128], BF16, tag=f"wv2{k}{m}", name=f"wv2{k}{m}") for m in range(2)] for k in range(2)]
        wfin_t = [[cst.tile([128, 128], BF16, tag=f"wf{k}{m}", name=f"wf{k}{m}") for m in range(4)] for k in range(2)]
        for k in range(2):
            nc.scalar.dma_start(wk2_t[k][:], wk2_d.ap()[k * 128:(k + 1) * 128, :])
            for m in range(2):
                nc.scalar.dma_start(wv2_t[k][m][:], wv2_d.ap()[k * 128:(k + 1) * 128, m * 128:(m + 1) * 128])
            for m in range(4):
                nc.scalar.dma_start(wfin_t[k][m][:], wfin_d.ap()[k * 128:(k + 1) * 128, m * 128:(m + 1) * 128])
        bnsc_t = cst.tile([128, 4], F32, tag="bnsc")
        bnbi_t = cst.tile([128, 4], F32, tag="bnbi")
        nc.sync.dma_start(bnsc_t[:], bnsc_d.ap().rearrange("(m p) -> p m", p=128))
        nc.sync.dma_start(bnbi_t[:], bnbi_d.ap().rearrange("(m p) -> p m", p=128))

        # ---------------- persistent: A accumulation, xs, stats ----------------
        A1 = [per.tile([128, H, W], BF16, tag=f"A1{i}", name=f"A1{i}") for i in range(2)]  # (c, h, w)
        xs_t = [per.tile([128, HS, HS], BF16, tag=f"xs{i}", name=f"xs{i}") for i in range(4)]
        s1p = per.tile([128, 4, 24], F32, tag="s1p")
        s2p = per.tile([128, 4, 24], F32, tag="s2p")

        # ---------------- attention scratch (closes before y phase) ----------
        atta_cm = tc.tile_pool(name="atta", bufs=1)
        atta = atta_cm.__enter__()
        qr_t = atta.tile([128, H, W], BF16, tag="qr_t")   # q' @0:32 and @64:96
        kk_t = atta.tile([64, H, W], BF16, tag="kk_t")    # k' x2 quadrants; k2 overwrites k1 in place
        v1 = [atta.tile([128, H, W], BF16, tag=f"v1{i}", name=f"v1{i}") for i in range(2)]  # (c, h, w); becomes v2 in place

        # ---------------- Phase 1+3: high stream on small grid, upsample -------
        with tc.tile_pool(name="ph13", bufs=1) as ph:
            wkvx_t = [[ph.tile([128, 128], BF16, tag=f"wkvx{m}_{k}", name=f"wkvx{m}_{k}") for k in range(4)] for m in range(7)]
            wkvx_q = [nc.sync, nc.scalar]
            for m in range(7):
                mw = min(128, NKV - m * 128)
                for k in range(4):
                    wkvx_q[(m * 4 + k) % 2].dma_start(wkvx_t[m][k][:, 0:mw],
                                        wkvx_d.ap()[k * 128:(k + 1) * 128, m * 128:m * 128 + mw])

            vs_t = [ph.tile([128, HS, HS], BF16, tag=f"vs{i}", name=f"vs{i}") for i in range(2)]
            ks_t = ph.tile([2 * CI, HS, HS], BF16, tag="ks")

            for n0 in range(0, PIXS, 512):
                nn = min(512, PIXS - n0)
                hi_c = [strm.tile([128, 512], BF16, tag=f"hic{k}", name=f"hic{k}", bufs=2) for k in range(4)]
                for k in range(4):
                    nc.gpsimd.dma_start(hi_c[k][:, 0:nn],
                                        high_d.ap().rearrange("c a b -> c (a b)")[k * 128:(k + 1) * 128, n0:n0 + nn])
                for m in range(7):
                    mw = min(128, NKV - m * 128)
                    pm = pb.tile([128, 512], F32, tag="pmm")
                    for k in range(4):
                        nc.tensor.matmul(pm[0:mw, 0:nn], wkvx_t[m][k][:, 0:mw],
                                         hi_c[k][:, 0:nn], start=(k == 0), stop=(k == 3))
                    if m < 2:
                        dst = vs_t[m][:].rearrange("c a b -> c (a b)")[:, n0:n0 + nn]
                    elif m < 6:
                        dst = xs_t[m - 2][:].rearrange("c a b -> c (a b)")[:, n0:n0 + nn]
                    else:
                        dst = ks_t[:].rearrange("c a b -> c (a b)")[:, n0:n0 + nn]
                    nc.scalar.activation(dst, pm[0:mw, 0:nn], AF.Copy)

            # upsample k1 (into qk[32:64]) and v1
            kw_t = ph.tile([2 * CI, HS, W], BF16, tag="kw")
            _up_last(nc, nc.vector, kw_t[:], ks_t[:])
            _up_mid(nc, nc.vector, kk_t[0:64], kw_t[:])
            for ct in range(2):
                vw_t = ph.tile([128, HS, W], BF16, tag="vw", name="vw", bufs=1)
                _up_last(nc, nc.vector, vw_t[:], vs_t[ct][:])
                _up_mid(nc, nc.vector, v1[ct][:], vw_t[:])

        # ---------------- round-only scratch ----------------
        attb_cm = tc.tile_pool(name="attb", bufs=1)
        attb = attb_cm.__enter__()
        expH = attb.tile([96, W, 96], BF16, tag="expH")   # [H', w, h]
        expW = attb.tile([96, H, 96], BF16, tag="expW")   # [W', h, w]

        # ---------------- Phase 2: q from low ----------------
        for n0 in range(0, PIX, 512):
            low_c = [strm.tile([128, 512], BF16, tag=f"hic{k}", name=f"lowc{k}", bufs=2) for k in range(4)]
            for k in range(4):
                nc.gpsimd.dma_start(low_c[k][:],
                                    low_d.ap().rearrange("c a b -> c (a b)")[k * 128:(k + 1) * 128, n0:n0 + 512])
            pm = pb.tile([128, 512], F32, tag="pmm", name="pmq")
            for k in range(4):
                nc.tensor.matmul(pm[:], wq_t[k][:], low_c[k][:], start=(k == 0), stop=(k == 3))
            nc.scalar.activation(qr_t[:].rearrange("c a b -> c (a b)")[:, n0:n0 + 512], pm[:], AF.Copy)

        # ---------------- attention helpers ----------------
        def energies(kbase):
            for w0 in range(0, W, 4):
                pes = pe.tile([96, 4, 96], F32, tag="pe")
                for j in range(4):
                    w = w0 + j
                    nc.tensor.matmul(pes[:, j, :], kk_t[0:32, :, w], qr_t[0:32, :, w],
                                     start=True, stop=True)
                nc.scalar.activation(expH[:, w0:w0 + 4, :], pes[:], AF.Exp)
                nc.gpsimd.tensor_mul(expH[:, w0:w0 + 4, :], expH[:, w0:w0 + 4, :], dmsk_t[:])
            for h0 in range(0, H, 4):
                pes = pe.tile([96, 4, 96], F32, tag="pe")
                for j in range(4):
                    h = h0 + j
                    nc.tensor.matmul(pes[:, j, :], kk_t[0:32, h, :], qr_t[0:32, h, :],
                                     start=True, stop=True)
                nc.scalar.activation(expW[:, h0:h0 + 4, :], pes[:], AF.Exp)

        def softmax_norm():
            expWv = expW[:].rearrange("p h w -> p w h")
            for w0 in range(0, W, 4):
                pss = ps.tile([128, 4, 96], F32, tag="ps")
                nc.tensor.matmul(pss[:], ones_t[:], expH[:, w0:w0 + 4, :], start=True, stop=False)
                nc.tensor.matmul(pss[:], ones_t[:], expWv[:, w0:w0 + 4, :], start=False, stop=True)
                srec = strm.tile([128, 4, 96], F32, tag="srec")
                nc.vector.reciprocal_approx_fast(srec[:], pss[:])
                nc.vector.tensor_mul(expH[:, w0:w0 + 4, :], expH[:, w0:w0 + 4, :], srec[0:96])
                nc.gpsimd.tensor_mul(expWv[:, w0:w0 + 4, :], expWv[:, w0:w0 + 4, :], srec[0:96])

        def aggregate(rnd, v):
            # W direction first: per-row h, natural (c, h, w) dst
            for h0 in range(0, H, 4):
                vtc = strm.tile([96, 4, 256], BF16, tag="vtc")
                for j in range(4):
                    h = h0 + j
                    for ct in range(2):
                        pt = pe.tile([96, 128], BF16, tag="pe")
                        nc.tensor.transpose(pt[:], v[ct][:, h, :], id_t[:])
                        nc.scalar.activation(vtc[:, j, ct * 128:(ct + 1) * 128], pt[:], AF.Copy)
                for half in range(2):
                    pag = pa.tile([128, 4, 96], F32, tag=f"pa{half}")
                    for j in range(4):
                        nc.tensor.matmul(pag[:, j, :], vtc[:, j, half * 128:(half + 1) * 128],
                                         expW[:, h0 + j, :], start=True, stop=True)
                    if rnd == 0:
                        nc.scalar.activation(A1[half][:, h0:h0 + 4, :], pag[:], AF.Copy)
                    else:
                        nc.vector.scalar_tensor_tensor(A1[half][:, h0:h0 + 4, :], pag[:], 1.0,
                                                       A1[half][:, h0:h0 + 4, :], AL.mult, AL.add)
            # H direction: per-column w, strided (c, w, h) view dst
            for w0 in range(0, W, 4):
                vtc = strm.tile([96, 4, 256], BF16, tag="vtc")
                for j in range(4):
                    w = w0 + j
                    for ct in range(2):
                        pt = pe.tile([96, 128], BF16, tag="pe")
                        nc.tensor.transpose(pt[:], v[ct][:, :, w], id_t[:])
                        nc.scalar.activation(vtc[:, j, ct * 128:(ct + 1) * 128], pt[:], AF.Copy)
                for half in range(2):
                    pag = pa.tile([128, 4, 96], F32, tag=f"pa{half}")
                    for j in range(4):
                        nc.tensor.matmul(pag[:, j, :], vtc[:, j, half * 128:(half + 1) * 128],
                                         expH[:, w0 + j, :], start=True, stop=True)
                    dst = A1[half][:].rearrange("c h w -> c w h")[:, w0:w0 + 4, :]
                    nc.vector.scalar_tensor_tensor(dst, pag[:], 1.0, dst, AL.mult, AL.add)

        # ---------------- round 1 ----------------
        energies(0)
        softmax_norm()
        aggregate(0, v1)

        # ---------------- round 2 prep (h-strips, all natural) ----------------
        for h0 in range(0, H, 4):
            pm = pb.tile([2 * CI, 4, 96], F32, tag="pmm")
            for k in range(2):
                nc.tensor.matmul(pm[:].rearrange("c a b -> c (a b)"), wk2_t[k][:],
                                 A1[k][:].rearrange("c h w -> c (h w)")[:, h0 * 96:(h0 + 4) * 96],
                                 start=(k == 0), stop=(k == 1))
            nc.vector.scalar_tensor_tensor(kk_t[0:64, h0:h0 + 4, :], pm[:], 1.0,
                                           kk_t[0:64, h0:h0 + 4, :], AL.mult, AL.add)
        for h0 in range(0, H, 4):
            for m in range(2):
                pm = pb.tile([128, 4, 96], F32, tag="pmm")
                for k in range(2):
                    nc.tensor.matmul(pm[:].rearrange("c a b -> c (a b)"), wv2_t[k][m][:],
                                     A1[k][:].rearrange("c h w -> c (h w)")[:, h0 * 96:(h0 + 4) * 96],
                                     start=(k == 0), stop=(k == 1))
                nc.vector.scalar_tensor_tensor(v1[m][:, h0:h0 + 4, :], pm[:], 1.0,
                                               v1[m][:, h0:h0 + 4, :], AL.mult, AL.add)

        # ---------------- round 2 ----------------
        energies(0)
        softmax_norm()
        aggregate(1, v1)

        if taps:
            nc.gpsimd.dma_start(taps["q"].ap().rearrange("c a b -> c (a b)"),
                              qr_t[0:32].rearrange("c a b -> c (a b)"))
            nc.gpsimd.dma_start(taps["k1"].ap().rearrange("c a b -> c (a b)"),
                              kk_t[0:32].rearrange("c a b -> c (a b)"))
            nc.gpsimd.dma_start(taps["k2"].ap().rearrange("c a b -> c (a b)"),
                              kk_t[0:32].rearrange("c a b -> c (a b)"))
            for ct in range(2):
                nc.gpsimd.dma_start(taps["v1"].ap().rearrange("c a b -> c (a b)")[ct * 128:(ct + 1) * 128, :],
                                  v1[ct][:].rearrange("c a b -> c (a b)"))
            nc.gpsimd.dma_start(taps["expH"].ap().rearrange("c a b -> c (a b)"),
                              expH[:].rearrange("c a b -> c (a b)"))

        attb_cm.__exit__(None, None, None)
        atta_cm.__exit__(None, None, None)

        # ---------------- final y (SBUF-resident) + stats ----------------
        yp_cm = tc.tile_pool(name="yp", bufs=1)
        yp = yp_cm.__enter__()
        ybig = [yp.tile([128, H, W], BF16, tag=f"yb{m}", name=f"yb{m}") for m in range(4)]

        def final_m(m):
            xw_t = yp.tile([128, HS, W], BF16, tag="xw", name="xw", bufs=1)
            _up_last(nc, nc.vector, xw_t[:], xs_t[m][:])
            x1m = yp.tile([128, H, W], BF16, tag="x1m", name="x1m", bufs=1)
            _up_mid(nc, nc.vector, x1m[:], xw_t[:])
            for hi, h0 in enumerate(range(0, H, 4)):
                pm = pa.tile([128, 4, 96], F32, tag=f"pa{hi % 2}")
                for k in range(2):
                    nc.tensor.matmul(pm[:].rearrange("c a b -> c (a b)"), wfin_t[k][m][:],
                                     A1[k][:].rearrange("c h w -> c (h w)")[:, h0 * 96:(h0 + 4) * 96],
                                     start=(k == 0), stop=(k == 1))
                nc.vector.scalar_tensor_tensor(ybig[m][:, h0:h0 + 4, :], pm[:], 1.0,
                                               x1m[:, h0:h0 + 4, :], AL.mult, AL.add,
                                               accum_out=s1p[:, m, hi].unsqueeze(1))
                junk = strm.tile([128, 4, 96], BF16, tag="junk")
                nc.scalar.activation(junk[:], ybig[m][:, h0:h0 + 4, :], AF.Square,
                                     accum_out=s2p[:, m, hi].unsqueeze(1))

        def stats_m(g):
            # stats AllReduce for m-tile g; returns (a, b) [128, 1]
            st_t = per.tile([128, 2], F32, tag=f"st{g}")
            nc.vector.tensor_reduce(st_t[:, 0:1], s1p[:, g:g + 1, :], mybir.AxisListType.X, AL.add)
            nc.vector.tensor_reduce(st_t[:, 1:2], s2p[:, g:g + 1, :], mybir.AxisListType.X, AL.add)
            nc.sync.dma_start(st_d[g].ap()[:], st_t[:])
            nc.gpsimd.collective_compute("AllReduce", AL.add, ins=[st_d[g].ap()], outs=[stg_d[g].ap()],
                                         replica_groups=[list(range(N_CORES))])
            stg_t = per.tile([128, 2], F32, tag=f"stg{g}")
            nc.sync.dma_start(stg_t[:], stg_d[g].ap()[:])
            mean_t = per.tile([128, 1], F32, tag=f"mean{g}")
            var_t = per.tile([128, 1], F32, tag=f"var{g}")
            nc.vector.tensor_scalar_mul(mean_t[:], stg_t[:, 0:1], 1.0 / NTOT)
            nc.vector.tensor_scalar_mul(var_t[:], stg_t[:, 1:2], 1.0 / NTOT)
            m2_t = per.tile([128, 1], F32, tag=f"m2{g}")
            nc.vector.tensor_mul(m2_t[:], mean_t[:], mean_t[:])
            nc.vector.tensor_sub(var_t[:], var_t[:], m2_t[:])
            sd_t = per.tile([128, 1], F32, tag=f"sd{g}")
            nc.scalar.activation(sd_t[:], var_t[:], AF.Sqrt, bias=eps_t[:, 0:1])
            ri_t = per.tile([128, 1], F32, tag=f"ri{g}")
            nc.vector.reciprocal(ri_t[:], sd_t[:])
            a_t = per.tile([128, 1], F32, tag=f"abn{g}")
            b_t = per.tile([128, 1], F32, tag=f"bbn{g}")
            nc.vector.tensor_mul(a_t[:], ri_t[:], bnsc_t[:, g:g + 1])
            nc.vector.tensor_mul(b_t[:], a_t[:], mean_t[:])
            nc.vector.tensor_sub(b_t[:], bnbi_t[:, g:g + 1], b_t[:])
            return a_t, b_t

        def apply_m(m, a_t, b_t):
            for hi, h0 in enumerate(range(0, H, 12)):
                oc = strm.tile([128, 12, 96], F32, tag="obn", bufs=3)
                nc.scalar.activation(oc[:], ybig[m][:, h0:h0 + 12, :], AF.Relu,
                                     scale=a_t[:, 0:1], bias=b_t[:, 0:1])
                eng = [nc.sync, nc.gpsimd][(m * 8 + hi) % 2]
                eng.dma_start(out_d.ap()[m * 128:(m + 1) * 128, h0:h0 + 12, :], oc[:])

        ab = {}
        for m in range(4):
            final_m(m)
            ab[m] = stats_m(m)   # each collective overlaps the next tiles' compute
        for m in range(4):
            apply_m(m, *ab[m])

        # ---------------- debug taps ----------------
        if taps:
            for ct in range(2):
                nc.gpsimd.dma_start(taps["A1"].ap().rearrange("c a b -> c (a b)")[ct * 128:(ct + 1) * 128, :],
                                  A1[ct][:].rearrange("c a b -> c (a b)"))
            for m in range(4):
                nc.gpsimd.dma_start(taps["y"].ap().rearrange("c a b -> c (a b)")[m * 128:(m + 1) * 128, :],
                                  ybig[m][:].rearrange("c a b -> c (a b)"))

        yp_cm.__exit__(None, None, None)

    nc.compile()
    return nc


# ---------------------------------------------------------------------------
# host entry
# ---------------------------------------------------------------------------

def _host_prep(inputs):
    conv1_w = _f32(inputs["conv1_w"]); conv2_w = _f32(inputs["conv2_w"])
    q_w = _f32(inputs["q_w"]); k_w = _f32(inputs["k_w"]); v_w = _f32(inputs["v_w"])
    gamma = float(np.asarray(inputs["gamma"]))
    wb = _f32(inputs["bottleneck_w"])
    wb_v, wb_h = wb[:, :CIN], wb[:, CIN:]

    wq = (q_w @ conv1_w) / 16.0
    wvs = v_w @ conv2_w
    wxs = (wb_v @ conv2_w + wb_h) / 16.0
    wks = k_w @ conv2_w
    wkvx = np.concatenate([wvs, wxs, wks, wks], axis=0)

    def _bf16(x):
        return np.ascontiguousarray(np.asarray(x, dtype=ml_dtypes.bfloat16))

    return {
        "wqT": _bf16(np.tile(wq.T, (1, 4))),
        "wkvxT": _bf16(wkvx.T),
        "wk2T": _bf16(np.tile((gamma * k_w).T, (1, 2))),
        "wv2T": _bf16((gamma * v_w).T),
        "wfinT": _bf16((gamma / 16.0 * wb_v).T),
        "bnsc": _f32(inputs["bn_scale"]),
        "bnbi": _f32(inputs["bn_bias"]),
    }


def _get_nc(debug_taps=False):
    key = ("nc", debug_taps)
    if key not in _CACHE:
        _CACHE[key] = build(debug_taps)
    return _CACHE[key]


def run(inputs, debug_taps=False, trace=False):
    for bname in ("conv1_b", "conv2_b", "q_b", "k_b", "v_b"):
        assert np.abs(np.asarray(inputs[bname])).max() == 0.0, f"nonzero {bname} unsupported"
    shared = _host_prep(inputs)
    low = _f32(inputs["low_feature"])
    high = _f32(inputs["high_feature"])
    in_maps = [dict(shared, low=low[i], high=high[i]) for i in range(N_CORES)]
    nc = _get_nc(debug_taps)
    res = run_bass_kernel_spmd(nc, in_maps, core_ids=list(range(N_CORES)), trace=trace)
    return res


def kernel(**inputs):
    res = run(inputs)
    out = np.stack([res.results[i]["out"] for i in range(N_CORES)], axis=0)
    return out.astype(np.float32)


# revision 29
# speedup vs baseline: 1.0118x; 1.0118x over previous
"""Trainium2 Bass kernel for nn_CCCrossLayerAttentionB (criss-cross cross-layer attention).

Self-contained: kernel(**inputs) -> np.ndarray [8, 512, 96, 96] fp32.

Sharding: data-parallel over batch (8 images -> 8 cores); BN stats via AllReduce.

Host-side restructuring (exact up to float assoc):
  - qf never materialized: q = (q_w @ conv1_w) @ low.
  - hf never materialized: every hf-consumer is a 1x1 conv, which commutes with the
    (separable, linear) bilinear 2x upsample, so the high stream is computed on the
    48x48 grid and upsampled afterward:
      ks = (k_w@conv2_w)@high; vs = (v_w@conv2_w)@high; xs = ((Wb_v@conv2_w+Wb_h)/16)@high
  - Device blends compute up16() = 16*up() in one fused op per output; the 1/16 is
    folded into host weights (q /16; xs /16; final att weights gamma/16).
  - vf2 never materialized:  k2' = (gamma k_w)@A1' + k1',  v2' = (gamma v_w)@A1' + v1',
    y = (gamma/16 Wb_v)@(A1'+A2') + up16(xs);  A' accumulated in place, h-major (c,h,w).
Attention per column w (and symmetrically per row h):
  e[H',h] = k'[:,H',w]^T q'[:,h,w] (flipped => softmax sums via ones-matmul, aggregation
  needs no attention transpose);  exp via ACT (+ -1e9 diag mask for the H direction);
  normalize exp tiles; aggregate with pixel-major v slices obtained from per-column
  PE transposes.
y never leaves SBUF: accumulated per-strip into resident (c,h,w) tiles with running
per-channel sum / sum-of-squares; after the stats AllReduce, BN+ReLU is applied from
SBUF and the output leaves as whole contiguous (c, h, w) blocks.
"""
import numpy as np
import ml_dtypes

import concourse.bass as bass
import concourse.bacc as bacc
import concourse.tile as tile
from concourse import mybir
from concourse.bass_utils import run_bass_kernel_spmd

F32 = mybir.dt.float32
BF16 = mybir.dt.bfloat16
AL = mybir.AluOpType
AF = mybir.ActivationFunctionType

N_CORES = 8
B, C, H, W = 8, 512, 96, 96
HS = 48
PIX = H * W
PIXS = HS * HS
CIN = 256
CI = 32
NTOT = float(B * PIX)
BN_EPS = 1e-5
NEG = -1e9

_CACHE = {}


def _f32(x):
    return np.ascontiguousarray(np.asarray(x, dtype=np.float32))


# ---------------------------------------------------------------------------
# blend helpers: up16 along last dim (48 -> 96) / middle dim
# ---------------------------------------------------------------------------

def _up_last(nc, eng, out, xin, eng2=None):
    """xin [P, n, 48] -> out [P, n, 96], out = 16 * bilinear (x4 per axis)."""
    eng2 = eng2 or eng
    eng.tensor_scalar_mul(out[:, :, 0:1], xin[:, :, 0:1], 4.0)
    eng.tensor_scalar_mul(out[:, :, 95:96], xin[:, :, 47:48], 4.0)
    eng.scalar_tensor_tensor(out[:, :, 2:95:2], xin[:, :, 1:48], 3.0, xin[:, :, 0:47],
                             AL.mult, AL.add)
    eng2.scalar_tensor_tensor(out[:, :, 1:94:2], xin[:, :, 0:47], 3.0, xin[:, :, 1:48],
                              AL.mult, AL.add)


def _up_mid(nc, eng, out, xin, eng2=None):
    """xin [P, 48, n] -> out [P, 96, n]."""
    eng2 = eng2 or eng
    eng.tensor_scalar_mul(out[:, 0:1, :], xin[:, 0:1, :], 4.0)
    eng.tensor_scalar_mul(out[:, 95:96, :], xin[:, 47:48, :], 4.0)
    eng.scalar_tensor_tensor(out[:, 2:95:2, :], xin[:, 1:48, :], 3.0, xin[:, 0:47, :],
                             AL.mult, AL.add)
    eng2.scalar_tensor_tensor(out[:, 1:94:2, :], xin[:, 0:47, :], 3.0, xin[:, 1:48, :],
                              AL.mult, AL.add)


# ---------------------------------------------------------------------------
# device kernel
# ---------------------------------------------------------------------------

def build(debug_taps=False):
    nc = bacc.Bacc("TRN2", target_bir_lowering=False, debug=False, num_devices=N_CORES)

    low_d = nc.dram_tensor("low", [C, H, W], F32, kind="ExternalInput")
    high_d = nc.dram_tensor("high", [C, HS, HS], F32, kind="ExternalInput")
    wq_d = nc.dram_tensor("wqT", [C, 128], BF16, kind="ExternalInput")
    wkvx_d = nc.dram_tensor("wkvxT", [C, CIN + C + 2 * CI], BF16, kind="ExternalInput")
    wk2_d = nc.dram_tensor("wk2T", [CIN, 2 * CI], BF16, kind="ExternalInput")
    wv2_d = nc.dram_tensor("wv2T", [CIN, CIN], BF16, kind="ExternalInput")
    wfin_d = nc.dram_tensor("wfinT", [CIN, C], BF16, kind="ExternalInput")
    bnsc_d = nc.dram_tensor("bnsc", [C], F32, kind="ExternalInput")
    bnbi_d = nc.dram_tensor("bnbi", [C], F32, kind="ExternalInput")
    out_d = nc.dram_tensor("out", [C, H, W], F32, kind="ExternalOutput")

    taps = {}
    if debug_taps:
        for nm, shp in [("q", [CI, H, W]), ("k1", [CI, H, W]), ("v1", [CIN, H, W]),
                        ("expH", [96, W, 96]), ("A1", [CIN, H, W]), ("y", [C, H, W]),
                        ("k2", [CI, H, W])]:
            taps[nm] = nc.dram_tensor("t_" + nm, shp, F32, kind="ExternalOutput")

    ident = nc.inline_tensor(np.eye(128, dtype=ml_dtypes.bfloat16), "ident")
    ones_l = nc.inline_tensor(np.ones((96, 128), dtype=ml_dtypes.bfloat16), "ones_l")
    epsv = nc.inline_tensor(np.full((128, 1), BN_EPS, np.float32), "epsv")
    dmsk = np.ones((96, 4, 96), dtype=ml_dtypes.bfloat16)
    for _p in range(96):
        dmsk[_p, :, _p] = 0
    dmsk_c = nc.inline_tensor(dmsk.reshape(96, 384), "dmskc")

    st_d = [nc.dram_tensor(f"st{g}_i", [128, 2], F32) for g in range(4)]
    stg_d = [nc.dram_tensor(f"stg{g}_i", [128, 2], F32, addr_space="Shared") for g in range(4)]

    NKV = CIN + C + 2 * CI  # 832: [vs 0:256 | xs 256:768 | ks(x2) 768:832]

    with tile.TileContext(nc) as tc, (
        tc.tile_pool(name="cst", bufs=1)) as cst, (
        tc.tile_pool(name="per", bufs=1)) as per, (
        tc.tile_pool(name="strm", bufs=3)) as strm, (
        tc.tile_pool(name="pe", bufs=2, space="PSUM")) as pe, (
        tc.tile_pool(name="ps", bufs=2, space="PSUM")) as ps, (
        tc.tile_pool(name="pa", bufs=1, space="PSUM")) as pa, (
        tc.tile_pool(name="pb", bufs=2, space="PSUM")) as pb:

        # ---------------- consts & weights ----------------
        id_t = cst.tile([128, 128], BF16, tag="id")
        nc.sync.dma_start(id_t[:], ident.ap()[:])
        ones_t = cst.tile([96, 128], BF16, tag="ones")
        nc.sync.dma_start(ones_t[:], ones_l.ap()[:])
        eps_t = cst.tile([128, 1], F32, tag="eps")
        nc.sync.dma_start(eps_t[:], epsv.ap()[:])
        dmsk_t = cst.tile([96, 4, 96], BF16, tag="dmsk")
        nc.sync.dma_start(dmsk_t[:].rearrange("p a b -> p (a b)"), dmsk_c.ap()[:])

        wq_t = [cst.tile([128, 128], BF16, tag=f"wq{k}", name=f"wq{k}") for k in range(4)]
        for k in range(4):
            nc.sync.dma_start(wq_t[k][:], wq_d.ap()[k * 128:(k + 1) * 128, :])
        wk2_t = [cst.tile([128, 2 * CI], BF16, tag=f"wk2{k}", name=f"wk2{k}") for k in range(2)]
        wv2_t = [[cst.tile([128, 128], BF16, tag=f"wv2{k}{m}", name=f"wv2{k}{m}") for m in range(2)] for k in range(2)]
        wfin_t = [[cst.tile([128, 128], BF16, tag=f"wf{k}{m}", name=f"wf{k}{m}") for m in range(4)] for k in range(2)]
        for k in range(2):
            nc.scalar.dma_start(wk2_t[k][:], wk2_d.ap()[k * 128:(k + 1) * 128, :])
            for m in range(2):
                nc.scalar.dma_start(wv2_t[k][m][:], wv2_d.ap()[k * 128:(k + 1) * 128, m * 128:(m + 1) * 128])
            for m in range(4):
                nc.scalar.dma_start(wfin_t[k][m][:], wfin_d.ap()[k * 128:(k + 1) * 128, m * 128:(m + 1) * 128])
        bnsc_t = cst.tile([128, 4], F32, tag="bnsc")
        bnbi_t = cst.tile([128, 4], F32, tag="bnbi")
        nc.sync.dma_start(bnsc_t[:], bnsc_d.ap().rearrange("(m p) -> p m", p=128))
        nc.sync.dma_start(bnbi_t[:], bnbi_d.ap().rearrange("(m p) -> p m", p=128))

        # ---------------- persistent: A accumulation, xs, stats ----------------
        A1 = [per.tile([128, H, W], BF16, tag=f"A1{i}", name=f"A1{i}") for i in range(2)]  # (c, h, w)
        xs_t = [per.tile([128, HS, HS], BF16, tag=f"xs{i}", name=f"xs{i}") for i in range(4)]
        s1p = per.tile([128, 4, 24], F32, tag="s1p")
        s2p = per.tile([128, 4, 24], F32, tag="s2p")

        # ---------------- attention scratch (closes before y phase) ----------
        atta_cm = tc.tile_pool(name="atta", bufs=1)
        atta = atta_cm.__enter__()
        qr_t = atta.tile([128, H, W], BF16, tag="qr_t")   # q' @0:32 and @64:96
        kk_t = atta.tile([64, H, W], BF16, tag="kk_t")    # k' x2 quadrants; k2 overwrites k1 in place
        v1 = [atta.tile([128, H, W], BF16, tag=f"v1{i}", name=f"v1{i}") for i in range(2)]  # (c, h, w); becomes v2 in place

        # ---------------- Phase 1+3: high stream on small grid, upsample -------
        with tc.tile_pool(name="ph13", bufs=1) as ph:
            wkvx_t = [[ph.tile([128, 128], BF16, tag=f"wkvx{m}_{k}", name=f"wkvx{m}_{k}") for k in range(4)] for m in range(7)]
            wkvx_q = [nc.sync, nc.scalar]
            for m in range(7):
                mw = min(128, NKV - m * 128)
                for k in range(4):
                    wkvx_q[(m * 4 + k) % 2].dma_start(wkvx_t[m][k][:, 0:mw],
                                        wkvx_d.ap()[k * 128:(k + 1) * 128, m * 128:m * 128 + mw])

            vs_t = [ph.tile([128, HS, HS], BF16, tag=f"vs{i}", name=f"vs{i}") for i in range(2)]
            ks_t = ph.tile([2 * CI, HS, HS], BF16, tag="ks")

            for n0 in range(0, PIXS, 512):
                nn = min(512, PIXS - n0)
                hi_c = [strm.tile([128, 512], BF16, tag=f"hic{k}", name=f"hic{k}", bufs=2) for k in range(4)]
                for k in range(4):
                    nc.gpsimd.dma_start(hi_c[k][:, 0:nn],
                                        high_d.ap().rearrange("c a b -> c (a b)")[k * 128:(k + 1) * 128, n0:n0 + nn])
                for m in range(7):
                    mw = min(128, NKV - m * 128)
                    pm = pb.tile([128, 512], F32, tag="pmm")
                    for k in range(4):
                        nc.tensor.matmul(pm[0:mw, 0:nn], wkvx_t[m][k][:, 0:mw],
                                         hi_c[k][:, 0:nn], start=(k == 0), stop=(k == 3))
                    if m < 2:
                        dst = vs_t[m][:].rearrange("c a b -> c (a b)")[:, n0:n0 + nn]
                    elif m < 6:
                        dst = xs_t[m - 2][:].rearrange("c a b -> c (a b)")[:, n0:n0 + nn]
                    else:
                        dst = ks_t[:].rearrange("c a b -> c (a b)")[:, n0:n0 + nn]
                    nc.scalar.activation(dst, pm[0:mw, 0:nn], AF.Copy)

            # upsample k1 (into qk[32:64]) and v1
            kw_t = ph.tile([2 * CI, HS, W], BF16, tag="kw")
            _up_last(nc, nc.vector, kw_t[:], ks_t[:])
            _up_mid(nc, nc.vector, kk_t[0:64], kw_t[:])
            for ct in range(2):
                vw_t = ph.tile([128, HS, W], BF16, tag="vw", name="vw", bufs=1)
                _up_last(nc, nc.vector, vw_t[:], vs_t[ct][:])
                _up_mid(nc, nc.vector, v1[ct][:], vw_t[:])

        # ---------------- round-only scratch ----------------
        attb_cm = tc.tile_pool(name="attb", bufs=1)
        attb = attb_cm.__enter__()
        expH = attb.tile([96, W, 96], BF16, tag="expH")   # [H', w, h]
        expW = attb.tile([96, H, 96], BF16, tag="expW")   # [W', h, w]

        # ---------------- Phase 2: q from low ----------------
        for n0 in range(0, PIX, 512):
            low_c = [strm.tile([128, 512], BF16, tag=f"hic{k}", name=f"lowc{k}", bufs=2) for k in range(4)]
            for k in range(4):
                nc.gpsimd.dma_start(low_c[k][:],
                                    low_d.ap().rearrange("c a b -> c (a b)")[k * 128:(k + 1) * 128, n0:n0 + 512])
            pm = pb.tile([128, 512], F32, tag="pmm", name="pmq")
            for k in range(4):
                nc.tensor.matmul(pm[:], wq_t[k][:], low_c[k][:], start=(k == 0), stop=(k == 3))
            nc.scalar.activation(qr_t[:].rearrange("c a b -> c (a b)")[:, n0:n0 + 512], pm[:], AF.Copy)

        # ---------------- attention helpers ----------------
        def energies(kbase):
            for w0 in range(0, W, 4):
                pes = pe.tile([96, 4, 96], F32, tag="pe")
                for j in range(4):
                    w = w0 + j
                    nc.tensor.matmul(pes[:, j, :], kk_t[0:32, :, w], qr_t[0:32, :, w],
                                     start=True, stop=True)
                nc.scalar.activation(expH[:, w0:w0 + 4, :], pes[:], AF.Exp)
                nc.gpsimd.tensor_mul(expH[:, w0:w0 + 4, :], expH[:, w0:w0 + 4, :], dmsk_t[:])
            for h0 in range(0, H, 4):
                pes = pe.tile([96, 4, 96], F32, tag="pe")
                for j in range(4):
                    h = h0 + j
                    nc.tensor.matmul(pes[:, j, :], kk_t[0:32, h, :], qr_t[0:32, h, :],
                                     start=True, stop=True)
                nc.scalar.activation(expW[:, h0:h0 + 4, :], pes[:], AF.Exp)

        def softmax_norm():
            expWv = expW[:].rearrange("p h w -> p w h")
            for w0 in range(0, W, 4):
                pss = ps.tile([128, 4, 96], F32, tag="ps")
                nc.tensor.matmul(pss[:], ones_t[:], expH[:, w0:w0 + 4, :], start=True, stop=False)
                nc.tensor.matmul(pss[:], ones_t[:], expWv[:, w0:w0 + 4, :], start=False, stop=True)
                srec = strm.tile([128, 4, 96], F32, tag="srec")
                nc.vector.reciprocal_approx_fast(srec[:], pss[:])
                nc.vector.tensor_mul(expH[:, w0:w0 + 4, :], expH[:, w0:w0 + 4, :], srec[0:96])
                nc.gpsimd.tensor_mul(expWv[:, w0:w0 + 4, :], expWv[:, w0:w0 + 4, :], srec[0:96])

        def aggregate(rnd, v):
            # W direction first: per-row h, natural (c, h, w) dst
            for h0 in range(0, H, 4):
                vtc = strm.tile([96, 4, 256], BF16, tag="vtc")
                for j in range(4):
                    h = h0 + j
                    for ct in range(2):
                        pt = pe.tile([96, 128], BF16, tag="pe")
                        nc.tensor.transpose(pt[:], v[ct][:, h, :], id_t[:])
                        nc.scalar.activation(vtc[:, j, ct * 128:(ct + 1) * 128], pt[:], AF.Copy)
                for half in range(2):
                    pag = pa.tile([128, 4, 96], F32, tag=f"pa{half}")
                    for j in range(4):
                        nc.tensor.matmul(pag[:, j, :], vtc[:, j, half * 128:(half + 1) * 128],
                                         expW[:, h0 + j, :], start=True, stop=True)
                    if rnd == 0:
                        nc.scalar.activation(A1[half][:, h0:h0 + 4, :], pag[:], AF.Copy)
                    else:
                        nc.vector.scalar_tensor_tensor(A1[half][:, h0:h0 + 4, :], pag[:], 1.0,
                                                       A1[half][:, h0:h0 + 4, :], AL.mult, AL.add)
            # H direction: per-column w, strided (c, w, h) view dst
            for w0 in range(0, W, 4):
                vtc = strm.tile([96, 4, 256], BF16, tag="vtc")
                for j in range(4):
                    w = w0 + j
                    for ct in range(2):
                        pt = pe.tile([96, 128], BF16, tag="pe")
                        nc.tensor.transpose(pt[:], v[ct][:, :, w], id_t[:])
                        nc.scalar.activation(vtc[:, j, ct * 128:(ct + 1) * 128], pt[:], AF.Copy)
                for half in range(2):
                    pag = pa.tile([128, 4, 96], F32, tag=f"pa{half}")
                    for j in range(4):
                        nc.tensor.matmul(pag[:, j, :], vtc[:, j, half * 128:(half + 1) * 128],
                                         expH[:, w0 + j, :], start=True, stop=True)
                    dst = A1[half][:].rearrange("c h w -> c w h")[:, w0:w0 + 4, :]
                    nc.vector.scalar_tensor_tensor(dst, pag[:], 1.0, dst, AL.mult, AL.add)

        # ---------------- round 1 ----------------
        energies(0)
        softmax_norm()
        aggregate(0, v1)

        # ---------------- round 2 prep (h-strips, all natural) ----------------
        for h0 in range(0, H, 4):
            pm = pb.tile([2 * CI, 4, 96], F32, tag="pmm")
            for k in range(2):
                nc.tensor.matmul(pm[:].rearrange("c a b -> c (a b)"), wk2_t[k][:],
                                 A1[k][:].rearrange("c h w -> c (h w)")[:, h0 * 96:(h0 + 4) * 96],
                                 start=(k == 0), stop=(k == 1))
            nc.vector.scalar_tensor_tensor(kk_t[0:64, h0:h0 + 4, :], pm[:], 1.0,
                                           kk_t[0:64, h0:h0 + 4, :], AL.mult, AL.add)
        for h0 in range(0, H, 4):
            for m in range(2):
                pm = pb.tile([128, 4, 96], F32, tag="pmm")
                for k in range(2):
                    nc.tensor.matmul(pm[:].rearrange("c a b -> c (a b)"), wv2_t[k][m][:],
                                     A1[k][:].rearrange("c h w -> c (h w)")[:, h0 * 96:(h0 + 4) * 96],
                                     start=(k == 0), stop=(k == 1))
                nc.vector.scalar_tensor_tensor(v1[m][:, h0:h0 + 4, :], pm[:], 1.0,
                                               v1[m][:, h0:h0 + 4, :], AL.mult, AL.add)

        # ---------------- round 2 ----------------
        energies(0)
        softmax_norm()
        aggregate(1, v1)

        if taps:
            nc.gpsimd.dma_start(taps["q"].ap().rearrange("c a b -> c (a b)"),
                              qr_t[0:32].rearrange("c a b -> c (a b)"))
            nc.gpsimd.dma_start(taps["k1"].ap().rearrange("c a b -> c (a b)"),
                              kk_t[0:32].rearrange("c a b -> c (a b)"))
            nc.gpsimd.dma_start(taps["k2"].ap().rearrange("c a b -> c (a b)"),
                              kk_t[0:32].rearrange("c a b -> c (a b)"))
            for ct in range(2):
                nc.gpsimd.dma_start(taps["v1"].ap().rearrange("c a b -> c (a b)")[ct * 128:(ct + 1) * 128, :],
                                  v1[ct][:].rearrange("c a b -> c (a b)"))
            nc.gpsimd.dma_start(taps["expH"].ap().rearrange("c a b -> c (a b)"),
                              expH[:].rearrange("c a b -> c (a b)"))

        attb_cm.__exit__(None, None, None)
        atta_cm.__exit__(None, None, None)

        # ---------------- final y (SBUF-resident) + stats ----------------
        yp_cm = tc.tile_pool(name="yp", bufs=1)
        yp = yp_cm.__enter__()
        ybig = [yp.tile([128, H, W], BF16, tag=f"yb{m}", name=f"yb{m}") for m in range(4)]

        def final_m(m, weave=None):
            xw_t = yp.tile([128, HS, W], BF16, tag="xw", name="xw", bufs=1)
            _up_last(nc, nc.vector, xw_t[:], xs_t[m][:])
            x1m = yp.tile([128, H, W], BF16, tag="x1m", name="x1m", bufs=1)
            _up_mid(nc, nc.vector, x1m[:], xw_t[:])
            for hi, h0 in enumerate(range(0, H, 4)):
                pm = pa.tile([128, 4, 96], F32, tag=f"pa{hi % 2}")
                for k in range(2):
                    nc.tensor.matmul(pm[:].rearrange("c a b -> c (a b)"), wfin_t[k][m][:],
                                     A1[k][:].rearrange("c h w -> c (h w)")[:, h0 * 96:(h0 + 4) * 96],
                                     start=(k == 0), stop=(k == 1))
                nc.vector.scalar_tensor_tensor(ybig[m][:, h0:h0 + 4, :], pm[:], 1.0,
                                               x1m[:, h0:h0 + 4, :], AL.mult, AL.add,
                                               accum_out=s1p[:, m, hi].unsqueeze(1))
                junk = strm.tile([128, 4, 96], BF16, tag="junk")
                nc.scalar.activation(junk[:], ybig[m][:, h0:h0 + 4, :], AF.Square,
                                     accum_out=s2p[:, m, hi].unsqueeze(1))
                if weave and hi % 3 == 2:
                    weave.pop(0)()

        def stats_m(g):
            # stats AllReduce for m-tile g; returns (a, b) [128, 1]
            st_t = per.tile([128, 2], F32, tag=f"st{g}")
            nc.vector.tensor_reduce(st_t[:, 0:1], s1p[:, g:g + 1, :], mybir.AxisListType.X, AL.add)
            nc.vector.tensor_reduce(st_t[:, 1:2], s2p[:, g:g + 1, :], mybir.AxisListType.X, AL.add)
            nc.sync.dma_start(st_d[g].ap()[:], st_t[:])
            nc.gpsimd.collective_compute("AllReduce", AL.add, ins=[st_d[g].ap()], outs=[stg_d[g].ap()],
                                         replica_groups=[list(range(N_CORES))])
            stg_t = per.tile([128, 2], F32, tag=f"stg{g}")
            nc.sync.dma_start(stg_t[:], stg_d[g].ap()[:])
            mean_t = per.tile([128, 1], F32, tag=f"mean{g}")
            var_t = per.tile([128, 1], F32, tag=f"var{g}")
            nc.vector.tensor_scalar_mul(mean_t[:], stg_t[:, 0:1], 1.0 / NTOT)
            nc.vector.tensor_scalar_mul(var_t[:], stg_t[:, 1:2], 1.0 / NTOT)
            m2_t = per.tile([128, 1], F32, tag=f"m2{g}")
            nc.vector.tensor_mul(m2_t[:], mean_t[:], mean_t[:])
            nc.vector.tensor_sub(var_t[:], var_t[:], m2_t[:])
            sd_t = per.tile([128, 1], F32, tag=f"sd{g}")
            nc.scalar.activation(sd_t[:], var_t[:], AF.Sqrt, bias=eps_t[:, 0:1])
            ri_t = per.tile([128, 1], F32, tag=f"ri{g}")
            nc.vector.reciprocal(ri_t[:], sd_t[:])
            a_t = per.tile([128, 1], F32, tag=f"abn{g}")
            b_t = per.tile([128, 1], F32, tag=f"bbn{g}")
            nc.vector.tensor_mul(a_t[:], ri_t[:], bnsc_t[:, g:g + 1])
            nc.vector.tensor_mul(b_t[:], a_t[:], mean_t[:])
            nc.vector.tensor_sub(b_t[:], bnbi_t[:, g:g + 1], b_t[:])
            return a_t, b_t

        def apply_chunk(m, a_t, b_t, hi, h0):
            oc = strm.tile([128, 12, 96], F32, tag="obn", bufs=3)
            nc.scalar.activation(oc[:], ybig[m][:, h0:h0 + 12, :], AF.Relu,
                                 scale=a_t[:, 0:1], bias=b_t[:, 0:1])
            eng = [nc.sync, nc.gpsimd][(m * 8 + hi) % 2]
            eng.dma_start(out_d.ap()[m * 128:(m + 1) * 128, h0:h0 + 12, :], oc[:])

        def apply_chunks(m, a_t, b_t):
            return [
                (lambda hi=hi, h0=h0: apply_chunk(m, a_t, b_t, hi, h0))
                for hi, h0 in enumerate(range(0, H, 12))
            ]

        final_m(0)
        ab0 = stats_m(0)
        final_m(1)
        ab1 = stats_m(1)
        final_m(2, weave=apply_chunks(0, *ab0))   # m0 applies fill scalar gaps
        ab2 = stats_m(2)
        final_m(3, weave=apply_chunks(1, *ab1))
        ab3 = stats_m(3)
        for f in apply_chunks(2, *ab2) + apply_chunks(3, *ab3):
            f()

        # ---------------- debug taps ----------------
        if taps:
            for ct in range(2):
                nc.gpsimd.dma_start(taps["A1"].ap().rearrange("c a b -> c (a b)")[ct * 128:(ct + 1) * 128, :],
                                  A1[ct][:].rearrange("c a b -> c (a b)"))
            for m in range(4):
                nc.gpsimd.dma_start(taps["y"].ap().rearrange("c a b -> c (a b)")[m * 128:(m + 1) * 128, :],
                                  ybig[m][:].rearrange("c a b -> c (a b)"))

        yp_cm.__exit__(None, None, None)

    nc.compile()
    return nc


# ---------------------------------------------------------------------------
# host entry
# ---------------------------------------------------------------------------

def _host_prep(inputs):
    conv1_w = _f32(inputs["conv1_w"]); conv2_w = _f32(inputs["conv2_w"])
    q_w = _f32(inputs["q_w"]); k_w = _f32(inputs["k_w"]); v_w = _f32(inputs["v_w"])
    gamma = float(np.asarray(inputs["gamma"]))
    wb = _f32(inputs["bottleneck_w"])
    wb_v, wb_h = wb[:, :CIN], wb[:, CIN:]

    wq = (q_w @ conv1_w) / 16.0
    wvs = v_w @ conv2_w
    wxs = (wb_v @ conv2_w + wb_h) / 16.0
    wks = k_w @ conv2_w
    wkvx = np.concatenate([wvs, wxs, wks, wks], axis=0)

    def _bf16(x):
        return np.ascontiguousarray(np.asarray(x, dtype=ml_dtypes.bfloat16))

    return {
        "wqT": _bf16(np.tile(wq.T, (1, 4))),
        "wkvxT": _bf16(wkvx.T),
        "wk2T": _bf16(np.tile((gamma * k_w).T, (1, 2))),
        "wv2T": _bf16((gamma * v_w).T),
        "wfinT": _bf16((gamma / 16.0 * wb_v).T),
        "bnsc": _f32(inputs["bn_scale"]),
        "bnbi": _f32(inputs["bn_bias"]),
    }


def _get_nc(debug_taps=False):
    key = ("nc", debug_taps)
    if key not in _CACHE:
        _CACHE[key] = build(debug_taps)
    return _CACHE[key]


def run(inputs, debug_taps=False, trace=False):
    for bname in ("conv1_b", "conv2_b", "q_b", "k_b", "v_b"):
        assert np.abs(np.asarray(inputs[bname])).max() == 0.0, f"nonzero {bname} unsupported"
    shared = _host_prep(inputs)
    low = _f32(inputs["low_feature"])
    high = _f32(inputs["high_feature"])
    in_maps = [dict(shared, low=low[i], high=high[i]) for i in range(N_CORES)]
    nc = _get_nc(debug_taps)
    res = run_bass_kernel_spmd(nc, in_maps, core_ids=list(range(N_CORES)), trace=trace)
    return res


def kernel(**inputs):
    res = run(inputs)
    out = np.stack([res.results[i]["out"] for i in range(N_CORES)], axis=0)
    return out.astype(np.float32)


# revision 31
# speedup vs baseline: 1.0208x; 1.0089x over previous
"""Trainium2 Bass kernel for nn_CCCrossLayerAttentionB (criss-cross cross-layer attention).

Self-contained: kernel(**inputs) -> np.ndarray [8, 512, 96, 96] fp32.

Sharding: data-parallel over batch (8 images -> 8 cores); BN stats via AllReduce.

Host-side restructuring (exact up to float assoc):
  - qf never materialized: q = (q_w @ conv1_w) @ low.
  - hf never materialized: every hf-consumer is a 1x1 conv, which commutes with the
    (separable, linear) bilinear 2x upsample, so the high stream is computed on the
    48x48 grid and upsampled afterward:
      ks = (k_w@conv2_w)@high; vs = (v_w@conv2_w)@high; xs = ((Wb_v@conv2_w+Wb_h)/16)@high
  - Device blends compute up16() = 16*up() in one fused op per output; the 1/16 is
    folded into host weights (q /16; xs /16; final att weights gamma/16).
  - vf2 never materialized:  k2' = (gamma k_w)@A1' + k1',  v2' = (gamma v_w)@A1' + v1',
    y = (gamma/16 Wb_v)@(A1'+A2') + up16(xs);  A' accumulated in place, h-major (c,h,w).
Attention per column w (and symmetrically per row h):
  e[H',h] = k'[:,H',w]^T q'[:,h,w] (flipped => softmax sums via ones-matmul, aggregation
  needs no attention transpose);  exp via ACT (+ -1e9 diag mask for the H direction);
  normalize exp tiles; aggregate with pixel-major v slices obtained from per-column
  PE transposes.
y never leaves SBUF: accumulated per-strip into resident (c,h,w) tiles with running
per-channel sum / sum-of-squares; after the stats AllReduce, BN+ReLU is applied from
SBUF and the output leaves as whole contiguous (c, h, w) blocks.
"""
import numpy as np
import ml_dtypes

import concourse.bass as bass
import concourse.bacc as bacc
import concourse.tile as tile
from concourse import mybir
from concourse.bass_utils import run_bass_kernel_spmd

F32 = mybir.dt.float32
BF16 = mybir.dt.bfloat16
AL = mybir.AluOpType
AF = mybir.ActivationFunctionType

N_CORES = 8
B, C, H, W = 8, 512, 96, 96
HS = 48
PIX = H * W
PIXS = HS * HS
CIN = 256
CI = 32
NTOT = float(B * PIX)
BN_EPS = 1e-5
NEG = -1e9

_CACHE = {}


def _f32(x):
    return np.ascontiguousarray(np.asarray(x, dtype=np.float32))


# ---------------------------------------------------------------------------
# blend helpers: up16 along last dim (48 -> 96) / middle dim
# ---------------------------------------------------------------------------

def _up_last(nc, eng, out, xin, eng2=None):
    """xin [P, n, 48] -> out [P, n, 96], out = 16 * bilinear (x4 per axis)."""
    eng2 = eng2 or eng
    eng.tensor_scalar_mul(out[:, :, 0:1], xin[:, :, 0:1], 4.0)
    eng.tensor_scalar_mul(out[:, :, 95:96], xin[:, :, 47:48], 4.0)
    eng.scalar_tensor_tensor(out[:, :, 2:95:2], xin[:, :, 1:48], 3.0, xin[:, :, 0:47],
                             AL.mult, AL.add)
    eng2.scalar_tensor_tensor(out[:, :, 1:94:2], xin[:, :, 0:47], 3.0, xin[:, :, 1:48],
                              AL.mult, AL.add)


def _up_mid(nc, eng, out, xin, eng2=None):
    """xin [P, 48, n] -> out [P, 96, n]."""
    eng2 = eng2 or eng
    eng.tensor_scalar_mul(out[:, 0:1, :], xin[:, 0:1, :], 4.0)
    eng.tensor_scalar_mul(out[:, 95:96, :], xin[:, 47:48, :], 4.0)
    eng.scalar_tensor_tensor(out[:, 2:95:2, :], xin[:, 1:48, :], 3.0, xin[:, 0:47, :],
                             AL.mult, AL.add)
    eng2.scalar_tensor_tensor(out[:, 1:94:2, :], xin[:, 0:47, :], 3.0, xin[:, 1:48, :],
                              AL.mult, AL.add)


# ---------------------------------------------------------------------------
# device kernel
# ---------------------------------------------------------------------------

def build(debug_taps=False):
    nc = bacc.Bacc("TRN2", target_bir_lowering=False, debug=False, num_devices=N_CORES)

    low_d = nc.dram_tensor("low", [C, H, W], F32, kind="ExternalInput")
    high_d = nc.dram_tensor("high", [C, HS, HS], F32, kind="ExternalInput")
    wq_d = nc.dram_tensor("wqT", [C, 128], BF16, kind="ExternalInput")
    wkvx_d = nc.dram_tensor("wkvxT", [C, CIN + C + 2 * CI], BF16, kind="ExternalInput")
    wk2_d = nc.dram_tensor("wk2T", [CIN, 2 * CI], BF16, kind="ExternalInput")
    wv2_d = nc.dram_tensor("wv2T", [CIN, CIN], BF16, kind="ExternalInput")
    wfin_d = nc.dram_tensor("wfinT", [CIN, C], BF16, kind="ExternalInput")
    bnsc_d = nc.dram_tensor("bnsc", [C], F32, kind="ExternalInput")
    bnbi_d = nc.dram_tensor("bnbi", [C], F32, kind="ExternalInput")
    out_d = nc.dram_tensor("out", [C, H, W], F32, kind="ExternalOutput")

    taps = {}
    if debug_taps:
        for nm, shp in [("q", [CI, H, W]), ("k1", [CI, H, W]), ("v1", [CIN, H, W]),
                        ("expH", [96, W, 96]), ("A1", [CIN, H, W]), ("y", [C, H, W]),
                        ("k2", [CI, H, W])]:
            taps[nm] = nc.dram_tensor("t_" + nm, shp, F32, kind="ExternalOutput")

    ident = nc.inline_tensor(np.eye(128, dtype=ml_dtypes.bfloat16), "ident")
    ones_l = nc.inline_tensor(np.ones((96, 128), dtype=ml_dtypes.bfloat16), "ones_l")
    epsv = nc.inline_tensor(np.full((128, 1), BN_EPS, np.float32), "epsv")
    dmsk = np.ones((96, 4, 96), dtype=ml_dtypes.bfloat16)
    for _p in range(96):
        dmsk[_p, :, _p] = 0
    dmsk_c = nc.inline_tensor(dmsk.reshape(96, 384), "dmskc")

    st_d = [nc.dram_tensor(f"st{g}_i", [128, 2], F32) for g in range(4)]
    stg_d = [nc.dram_tensor(f"stg{g}_i", [128, 2], F32, addr_space="Shared") for g in range(4)]

    NKV = CIN + C + 2 * CI  # 832: [vs 0:256 | xs 256:768 | ks(x2) 768:832]

    with tile.TileContext(nc) as tc, (
        tc.tile_pool(name="cst", bufs=1)) as cst, (
        tc.tile_pool(name="per", bufs=1)) as per, (
        tc.tile_pool(name="strm", bufs=3)) as strm, (
        tc.tile_pool(name="pe", bufs=2, space="PSUM")) as pe, (
        tc.tile_pool(name="ps", bufs=2, space="PSUM")) as ps, (
        tc.tile_pool(name="pa", bufs=1, space="PSUM")) as pa, (
        tc.tile_pool(name="pb", bufs=2, space="PSUM")) as pb:

        # ---------------- consts & weights ----------------
        id_t = cst.tile([128, 128], BF16, tag="id")
        nc.sync.dma_start(id_t[:], ident.ap()[:])
        ones_t = cst.tile([96, 128], BF16, tag="ones")
        nc.sync.dma_start(ones_t[:], ones_l.ap()[:])
        eps_t = cst.tile([128, 1], F32, tag="eps")
        nc.sync.dma_start(eps_t[:], epsv.ap()[:])
        dmsk_t = cst.tile([96, 4, 96], BF16, tag="dmsk")
        nc.sync.dma_start(dmsk_t[:].rearrange("p a b -> p (a b)"), dmsk_c.ap()[:])

        wq_t = [cst.tile([128, 128], BF16, tag=f"wq{k}", name=f"wq{k}") for k in range(4)]
        for k in range(4):
            nc.sync.dma_start(wq_t[k][:], wq_d.ap()[k * 128:(k + 1) * 128, :])
        wk2_t = [cst.tile([128, 2 * CI], BF16, tag=f"wk2{k}", name=f"wk2{k}") for k in range(2)]
        wv2_t = [[cst.tile([128, 128], BF16, tag=f"wv2{k}{m}", name=f"wv2{k}{m}") for m in range(2)] for k in range(2)]
        wfin_t = [[cst.tile([128, 128], BF16, tag=f"wf{k}{m}", name=f"wf{k}{m}") for m in range(4)] for k in range(2)]
        for k in range(2):
            nc.scalar.dma_start(wk2_t[k][:], wk2_d.ap()[k * 128:(k + 1) * 128, :])
            for m in range(2):
                nc.scalar.dma_start(wv2_t[k][m][:], wv2_d.ap()[k * 128:(k + 1) * 128, m * 128:(m + 1) * 128])
            for m in range(4):
                nc.scalar.dma_start(wfin_t[k][m][:], wfin_d.ap()[k * 128:(k + 1) * 128, m * 128:(m + 1) * 128])
        bnsc_t = cst.tile([128, 4], F32, tag="bnsc")
        bnbi_t = cst.tile([128, 4], F32, tag="bnbi")
        nc.sync.dma_start(bnsc_t[:], bnsc_d.ap().rearrange("(m p) -> p m", p=128))
        nc.sync.dma_start(bnbi_t[:], bnbi_d.ap().rearrange("(m p) -> p m", p=128))

        # ---------------- persistent: A accumulation, xs, stats ----------------
        A1 = [per.tile([128, H, W], BF16, tag=f"A1{i}", name=f"A1{i}") for i in range(2)]  # (c, h, w)
        xs_t = [per.tile([128, HS, HS], BF16, tag=f"xs{i}", name=f"xs{i}") for i in range(4)]
        s1p = per.tile([128, 4, 24], F32, tag="s1p")
        s2p = per.tile([128, 4, 24], F32, tag="s2p")

        # ---------------- attention scratch (closes before y phase) ----------
        atta_cm = tc.tile_pool(name="atta", bufs=1)
        atta = atta_cm.__enter__()
        qr_t = atta.tile([128, H, W], BF16, tag="qr_t")   # q' @0:32 and @64:96
        kk_t = atta.tile([64, H, W], BF16, tag="kk_t")    # k' x2 quadrants; k2 overwrites k1 in place
        v1 = [atta.tile([128, H, W], BF16, tag=f"v1{i}", name=f"v1{i}") for i in range(2)]  # (c, h, w); becomes v2 in place

        # ---------------- Phase 1+3: high stream on small grid, upsample -------
        with tc.tile_pool(name="ph13", bufs=1) as ph:
            wkvx_t = [[ph.tile([128, 128], BF16, tag=f"wkvx{m}_{k}", name=f"wkvx{m}_{k}") for k in range(4)] for m in range(7)]
            wkvx_q = [nc.sync, nc.scalar]
            for m in range(7):
                mw = min(128, NKV - m * 128)
                for k in range(4):
                    wkvx_q[(m * 4 + k) % 2].dma_start(wkvx_t[m][k][:, 0:mw],
                                        wkvx_d.ap()[k * 128:(k + 1) * 128, m * 128:m * 128 + mw])

            vs_t = [ph.tile([128, HS, HS], BF16, tag=f"vs{i}", name=f"vs{i}") for i in range(2)]
            ks_t = ph.tile([2 * CI, HS, HS], BF16, tag="ks")

            for n0 in range(0, PIXS, 512):
                nn = min(512, PIXS - n0)
                hi_c = [strm.tile([128, 512], BF16, tag=f"hic{k}", name=f"hic{k}", bufs=2) for k in range(4)]
                for k in range(4):
                    nc.gpsimd.dma_start(hi_c[k][:, 0:nn],
                                        high_d.ap().rearrange("c a b -> c (a b)")[k * 128:(k + 1) * 128, n0:n0 + nn])
                for m in range(7):
                    mw = min(128, NKV - m * 128)
                    pm = pb.tile([128, 512], F32, tag="pmm")
                    for k in range(4):
                        nc.tensor.matmul(pm[0:mw, 0:nn], wkvx_t[m][k][:, 0:mw],
                                         hi_c[k][:, 0:nn], start=(k == 0), stop=(k == 3))
                    if m < 2:
                        dst = vs_t[m][:].rearrange("c a b -> c (a b)")[:, n0:n0 + nn]
                    elif m < 6:
                        dst = xs_t[m - 2][:].rearrange("c a b -> c (a b)")[:, n0:n0 + nn]
                    else:
                        dst = ks_t[:].rearrange("c a b -> c (a b)")[:, n0:n0 + nn]
                    nc.scalar.activation(dst, pm[0:mw, 0:nn], AF.Copy)

            # upsample k1 (into qk[32:64]) and v1
            kw_t = ph.tile([2 * CI, HS, W], BF16, tag="kw")
            _up_last(nc, nc.vector, kw_t[:], ks_t[:])
            _up_mid(nc, nc.vector, kk_t[0:64], kw_t[:])
            for ct in range(2):
                vw_t = ph.tile([128, HS, W], BF16, tag="vw", name="vw", bufs=1)
                _up_last(nc, nc.vector, vw_t[:], vs_t[ct][:])
                _up_mid(nc, nc.vector, v1[ct][:], vw_t[:])

        # ---------------- round-only scratch ----------------
        attb_cm = tc.tile_pool(name="attb", bufs=1)
        attb = attb_cm.__enter__()
        expH = attb.tile([96, W, 96], BF16, tag="expH")   # [H', w, h]
        expW = attb.tile([96, H, 96], BF16, tag="expW")   # [W', h, w]

        # ---------------- Phase 2: q from low ----------------
        for n0 in range(0, PIX, 512):
            low_c = [strm.tile([128, 512], BF16, tag=f"hic{k}", name=f"lowc{k}", bufs=2) for k in range(4)]
            for k in range(4):
                nc.gpsimd.dma_start(low_c[k][:],
                                    low_d.ap().rearrange("c a b -> c (a b)")[k * 128:(k + 1) * 128, n0:n0 + 512])
            pm = pb.tile([128, 512], F32, tag="pmm", name="pmq")
            for k in range(4):
                nc.tensor.matmul(pm[:], wq_t[k][:], low_c[k][:], start=(k == 0), stop=(k == 3))
            nc.scalar.activation(qr_t[:].rearrange("c a b -> c (a b)")[:, n0:n0 + 512], pm[:], AF.Copy)

        # ---------------- attention helpers ----------------
        def energies(kbase):
            for w0 in range(0, W, 4):
                pes = pe.tile([96, 4, 96], F32, tag="pe")
                for j in range(4):
                    w = w0 + j
                    nc.tensor.matmul(pes[:, j, :], kk_t[0:32, :, w], qr_t[0:32, :, w],
                                     start=True, stop=True)
                nc.scalar.activation(expH[:, w0:w0 + 4, :], pes[:], AF.Exp)
                nc.gpsimd.tensor_mul(expH[:, w0:w0 + 4, :], expH[:, w0:w0 + 4, :], dmsk_t[:])
            for h0 in range(0, H, 4):
                pes = pe.tile([96, 4, 96], F32, tag="pe")
                for j in range(4):
                    h = h0 + j
                    nc.tensor.matmul(pes[:, j, :], kk_t[0:32, h, :], qr_t[0:32, h, :],
                                     start=True, stop=True)
                nc.scalar.activation(expW[:, h0:h0 + 4, :], pes[:], AF.Exp)

        def softmax_norm():
            expWv = expW[:].rearrange("p h w -> p w h")
            for w0 in range(0, W, 4):
                pss = ps.tile([128, 4, 96], F32, tag="ps")
                nc.tensor.matmul(pss[:], ones_t[:], expH[:, w0:w0 + 4, :], start=True, stop=False)
                nc.tensor.matmul(pss[:], ones_t[:], expWv[:, w0:w0 + 4, :], start=False, stop=True)
                srec = strm.tile([128, 4, 96], F32, tag="srec")
                nc.vector.reciprocal_approx_fast(srec[:], pss[:])
                nc.vector.tensor_mul(expH[:, w0:w0 + 4, :], expH[:, w0:w0 + 4, :], srec[0:96])
                nc.gpsimd.tensor_mul(expWv[:, w0:w0 + 4, :], expWv[:, w0:w0 + 4, :], srec[0:96])

        def aggregate(rnd, v):
            # W direction first: per-row h, natural (c, h, w) dst
            for h0 in range(0, H, 4):
                vtc = strm.tile([96, 4, 256], BF16, tag="vtc")
                for j in range(4):
                    h = h0 + j
                    for ct in range(2):
                        pt = pe.tile([96, 128], BF16, tag="pe")
                        nc.tensor.transpose(pt[:], v[ct][:, h, :], id_t[:])
                        nc.scalar.activation(vtc[:, j, ct * 128:(ct + 1) * 128], pt[:], AF.Copy)
                for half in range(2):
                    pag = pa.tile([128, 4, 96], F32, tag=f"pa{half}")
                    for j in range(4):
                        nc.tensor.matmul(pag[:, j, :], vtc[:, j, half * 128:(half + 1) * 128],
                                         expW[:, h0 + j, :], start=True, stop=True)
                    if rnd == 0:
                        nc.scalar.activation(A1[half][:, h0:h0 + 4, :], pag[:], AF.Copy)
                    else:
                        nc.vector.scalar_tensor_tensor(A1[half][:, h0:h0 + 4, :], pag[:], 1.0,
                                                       A1[half][:, h0:h0 + 4, :], AL.mult, AL.add)
            # H direction: per-column w, strided (c, w, h) view dst
            for w0 in range(0, W, 4):
                vtc = strm.tile([96, 4, 256], BF16, tag="vtc")
                for j in range(4):
                    w = w0 + j
                    for ct in range(2):
                        pt = pe.tile([96, 128], BF16, tag="pe")
                        nc.tensor.transpose(pt[:], v[ct][:, :, w], id_t[:])
                        nc.scalar.activation(vtc[:, j, ct * 128:(ct + 1) * 128], pt[:], AF.Copy)
                for half in range(2):
                    pag = pa.tile([128, 4, 96], F32, tag=f"pa{half}")
                    for j in range(4):
                        nc.tensor.matmul(pag[:, j, :], vtc[:, j, half * 128:(half + 1) * 128],
                                         expH[:, w0 + j, :], start=True, stop=True)
                    dst = A1[half][:].rearrange("c h w -> c w h")[:, w0:w0 + 4, :]
                    nc.vector.scalar_tensor_tensor(dst, pag[:], 1.0, dst, AL.mult, AL.add)

        # ---------------- round 1 ----------------
        energies(0)
        softmax_norm()
        aggregate(0, v1)

        # ---------------- round 2 prep (h-strips, all natural) ----------------
        for h0 in range(0, H, 4):
            pm = pb.tile([2 * CI, 4, 96], F32, tag="pmm")
            for k in range(2):
                nc.tensor.matmul(pm[:].rearrange("c a b -> c (a b)"), wk2_t[k][:],
                                 A1[k][:].rearrange("c h w -> c (h w)")[:, h0 * 96:(h0 + 4) * 96],
                                 start=(k == 0), stop=(k == 1))
            nc.vector.scalar_tensor_tensor(kk_t[0:64, h0:h0 + 4, :], pm[:], 1.0,
                                           kk_t[0:64, h0:h0 + 4, :], AL.mult, AL.add)
        for h0 in range(0, H, 4):
            for m in range(2):
                pm = pb.tile([128, 4, 96], F32, tag="pmm")
                for k in range(2):
                    nc.tensor.matmul(pm[:].rearrange("c a b -> c (a b)"), wv2_t[k][m][:],
                                     A1[k][:].rearrange("c h w -> c (h w)")[:, h0 * 96:(h0 + 4) * 96],
                                     start=(k == 0), stop=(k == 1))
                nc.vector.scalar_tensor_tensor(v1[m][:, h0:h0 + 4, :], pm[:], 1.0,
                                               v1[m][:, h0:h0 + 4, :], AL.mult, AL.add)

        # ---------------- round 2 ----------------
        energies(0)
        softmax_norm()
        aggregate(1, v1)

        if taps:
            nc.gpsimd.dma_start(taps["q"].ap().rearrange("c a b -> c (a b)"),
                              qr_t[0:32].rearrange("c a b -> c (a b)"))
            nc.gpsimd.dma_start(taps["k1"].ap().rearrange("c a b -> c (a b)"),
                              kk_t[0:32].rearrange("c a b -> c (a b)"))
            nc.gpsimd.dma_start(taps["k2"].ap().rearrange("c a b -> c (a b)"),
                              kk_t[0:32].rearrange("c a b -> c (a b)"))
            for ct in range(2):
                nc.gpsimd.dma_start(taps["v1"].ap().rearrange("c a b -> c (a b)")[ct * 128:(ct + 1) * 128, :],
                                  v1[ct][:].rearrange("c a b -> c (a b)"))
            nc.gpsimd.dma_start(taps["expH"].ap().rearrange("c a b -> c (a b)"),
                              expH[:].rearrange("c a b -> c (a b)"))

        attb_cm.__exit__(None, None, None)
        atta_cm.__exit__(None, None, None)

        # ---------------- final y (SBUF-resident) + stats ----------------
        yp_cm = tc.tile_pool(name="yp", bufs=1)
        yp = yp_cm.__enter__()
        ybig = [yp.tile([128, H, W], BF16, tag=f"yb{m}", name=f"yb{m}") for m in range(4)]

        def final_m(m, weave=None):
            xw_t = yp.tile([128, HS, W], BF16, tag="xw", name="xw", bufs=1)
            _up_last(nc, nc.vector, xw_t[:], xs_t[m][:])
            x1m = yp.tile([128, H, W], BF16, tag="x1m", name="x1m", bufs=1)
            _up_mid(nc, nc.vector, x1m[:], xw_t[:])
            for hi, h0 in enumerate(range(0, H, 4)):
                pm = pa.tile([128, 4, 96], F32, tag=f"pa{hi % 2}")
                for k in range(2):
                    nc.tensor.matmul(pm[:].rearrange("c a b -> c (a b)"), wfin_t[k][m][:],
                                     A1[k][:].rearrange("c h w -> c (h w)")[:, h0 * 96:(h0 + 4) * 96],
                                     start=(k == 0), stop=(k == 1))
                nc.vector.scalar_tensor_tensor(ybig[m][:, h0:h0 + 4, :], pm[:], 1.0,
                                               x1m[:, h0:h0 + 4, :], AL.mult, AL.add,
                                               accum_out=s1p[:, m, hi].unsqueeze(1))
                junk = strm.tile([128, 4, 96], BF16, tag="junk")
                nc.scalar.activation(junk[:], ybig[m][:, h0:h0 + 4, :], AF.Square,
                                     accum_out=s2p[:, m, hi].unsqueeze(1))
                if weave and hi % 3 == 2:
                    weave.pop(0)()

        def stats_m(g):
            # stats AllReduce for m-tile g; returns (a, b) [128, 1]
            st_t = per.tile([128, 2], F32, tag=f"st{g}")
            nc.vector.tensor_reduce(st_t[:, 0:1], s1p[:, g:g + 1, :], mybir.AxisListType.X, AL.add)
            nc.vector.tensor_reduce(st_t[:, 1:2], s2p[:, g:g + 1, :], mybir.AxisListType.X, AL.add)
            nc.sync.dma_start(st_d[g].ap()[:], st_t[:])
            nc.gpsimd.collective_compute("AllReduce", AL.add, ins=[st_d[g].ap()], outs=[stg_d[g].ap()],
                                         replica_groups=[list(range(N_CORES))])
            stg_t = per.tile([128, 2], F32, tag=f"stg{g}")
            nc.sync.dma_start(stg_t[:], stg_d[g].ap()[:])
            mean_t = per.tile([128, 1], F32, tag=f"mean{g}")
            var_t = per.tile([128, 1], F32, tag=f"var{g}")
            nc.vector.tensor_scalar_mul(mean_t[:], stg_t[:, 0:1], 1.0 / NTOT)
            nc.vector.tensor_scalar_mul(var_t[:], stg_t[:, 1:2], 1.0 / NTOT)
            m2_t = per.tile([128, 1], F32, tag=f"m2{g}")
            nc.vector.tensor_mul(m2_t[:], mean_t[:], mean_t[:])
            nc.vector.tensor_sub(var_t[:], var_t[:], m2_t[:])
            sd_t = per.tile([128, 1], F32, tag=f"sd{g}")
            nc.scalar.activation(sd_t[:], var_t[:], AF.Sqrt, bias=eps_t[:, 0:1])
            ri_t = per.tile([128, 1], F32, tag=f"ri{g}")
            nc.vector.reciprocal(ri_t[:], sd_t[:])
            a_t = per.tile([128, 1], F32, tag=f"abn{g}")
            b_t = per.tile([128, 1], F32, tag=f"bbn{g}")
            nc.vector.tensor_mul(a_t[:], ri_t[:], bnsc_t[:, g:g + 1])
            nc.vector.tensor_mul(b_t[:], a_t[:], mean_t[:])
            nc.vector.tensor_sub(b_t[:], bnbi_t[:, g:g + 1], b_t[:])
            return a_t, b_t

        def apply_chunk(m, a_t, b_t, hi, h0):
            oc = strm.tile([128, 12, 96], F32, tag="obn", bufs=3)
            nc.scalar.activation(oc[:], ybig[m][:, h0:h0 + 12, :], AF.Relu,
                                 scale=a_t[:, 0:1], bias=b_t[:, 0:1])
            eng = [nc.sync, nc.gpsimd][(m * 8 + hi) % 2]
            eng.dma_start(out_d.ap()[m * 128:(m + 1) * 128, h0:h0 + 12, :], oc[:])

        def apply_chunks(m, a_t, b_t):
            return [
                (lambda hi=hi, h0=h0: apply_chunk(m, a_t, b_t, hi, h0))
                for hi, h0 in enumerate(range(0, H, 12))
            ]

        final_m(0)
        ab0 = stats_m(0)
        final_m(1)
        ab1 = stats_m(1)
        final_m(2, weave=apply_chunks(0, *ab0))   # m0 applies fill scalar gaps
        ab2 = stats_m(2)
        final_m(3, weave=apply_chunks(1, *ab1))
        ab3 = stats_m(3)
        for f in apply_chunks(2, *ab2) + apply_chunks(3, *ab3):
            f()

        # ---------------- debug taps ----------------
        if taps:
            for ct in range(2):
                nc.gpsimd.dma_start(taps["A1"].ap().rearrange("c a b -> c (a b)")[ct * 128:(ct + 1) * 128, :],
                                  A1[ct][:].rearrange("c a b -> c (a b)"))
            for m in range(4):
                nc.gpsimd.dma_start(taps["y"].ap().rearrange("c a b -> c (a b)")[m * 128:(m + 1) * 128, :],
                                  ybig[m][:].rearrange("c a b -> c (a b)"))

        yp_cm.__exit__(None, None, None)

    nc.compile()
    return nc


# ---------------------------------------------------------------------------
# host entry
# ---------------------------------------------------------------------------

def _host_prep(inputs):
    conv1_w = _f32(inputs["conv1_w"]); conv2_w = _f32(inputs["conv2_w"])
    q_w = _f32(inputs["q_w"]); k_w = _f32(inputs["k_w"]); v_w = _f32(inputs["v_w"])
    gamma = float(np.asarray(inputs["gamma"]))
    wb = _f32(inputs["bottleneck_w"])
    wb_v, wb_h = wb[:, :CIN], wb[:, CIN:]

    wq = (q_w @ conv1_w) / 16.0
    wvs = v_w @ conv2_w
    wxs = (wb_v @ conv2_w + wb_h) / 16.0
    wks = k_w @ conv2_w
    wkvx = np.concatenate([wvs, wxs, wks, wks], axis=0)

    def _bf16(x):
        return np.ascontiguousarray(np.asarray(x, dtype=ml_dtypes.bfloat16))

    return {
        "wqT": _bf16(np.tile(wq.T, (1, 4))),
        "wkvxT": _bf16(wkvx.T),
        "wk2T": _bf16(np.tile((gamma * k_w).T, (1, 2))),
        "wv2T": _bf16((gamma * v_w).T),
        "wfinT": _bf16((gamma / 16.0 * wb_v).T),
        "bnsc": _f32(inputs["bn_scale"]),
        "bnbi": _f32(inputs["bn_bias"]),
    }


def _get_nc(debug_taps=False):
    key = ("nc", debug_taps)
    if key not in _CACHE:
        _CACHE[key] = build(debug_taps)
    return _CACHE[key]


def run(inputs, debug_taps=False, trace=False):
    for bname in ("conv1_b", "conv2_b", "q_b", "k_b", "v_b"):
        assert np.abs(np.asarray(inputs[bname])).max() == 0.0, f"nonzero {bname} unsupported"
    shared = _host_prep(inputs)
    low = _f32(inputs["low_feature"])
    high = _f32(inputs["high_feature"])
    in_maps = [dict(shared, low=low[i], high=high[i]) for i in range(N_CORES)]
    nc = _get_nc(debug_taps)
    res = run_bass_kernel_spmd(nc, in_maps, core_ids=list(range(N_CORES)), trace=trace)
    return res


def kernel(**inputs):
    res = run(inputs)
    out = np.stack([res.results[i]["out"] for i in range(N_CORES)], axis=0)
    return out.astype(np.float32)


# revision 32
# speedup vs baseline: 1.0887x; 1.0665x over previous
"""Trainium2 Bass kernel for nn_CCCrossLayerAttentionB (criss-cross cross-layer attention).

Self-contained: kernel(**inputs) -> np.ndarray [8, 512, 96, 96] fp32.

Sharding: data-parallel over batch (8 images -> 8 cores); BN stats via AllReduce.

Host-side restructuring (exact up to float assoc):
  - qf never materialized: q = (q_w @ conv1_w) @ low.
  - hf never materialized: every hf-consumer is a 1x1 conv, which commutes with the
    (separable, linear) bilinear 2x upsample, so the high stream is computed on the
    48x48 grid and upsampled afterward:
      ks = (k_w@conv2_w)@high; vs = (v_w@conv2_w)@high; xs = ((Wb_v@conv2_w+Wb_h)/16)@high
  - Device blends compute up16() = 16*up() in one fused op per output; the 1/16 is
    folded into host weights (q /16; xs /16; final att weights gamma/16).
  - vf2 never materialized:  k2' = (gamma k_w)@A1' + k1',  v2' = (gamma v_w)@A1' + v1',
    y = (gamma/16 Wb_v)@(A1'+A2') + up16(xs);  A' accumulated in place, h-major (c,h,w).
Attention per column w (and symmetrically per row h):
  e[H',h] = k'[:,H',w]^T q'[:,h,w] (flipped => softmax sums via ones-matmul, aggregation
  needs no attention transpose);  exp via ACT (+ -1e9 diag mask for the H direction);
  normalize exp tiles; aggregate with pixel-major v slices obtained from per-column
  PE transposes.
y never leaves SBUF: accumulated per-strip into resident (c,h,w) tiles with running
per-channel sum / sum-of-squares; after the stats AllReduce, BN+ReLU is applied from
SBUF and the output leaves as whole contiguous (c, h, w) blocks.
"""
import numpy as np
import ml_dtypes

import concourse.bass as bass
import concourse.bacc as bacc
import concourse.tile as tile
from concourse import mybir
from concourse.bass_utils import run_bass_kernel_spmd

F32 = mybir.dt.float32
BF16 = mybir.dt.bfloat16
AL = mybir.AluOpType
AF = mybir.ActivationFunctionType

N_CORES = 8
B, C, H, W = 8, 512, 96, 96
HS = 48
PIX = H * W
PIXS = HS * HS
CIN = 256
CI = 32
NTOT = float(B * PIX)
BN_EPS = 1e-5
NEG = -1e9

_CACHE = {}


def _f32(x):
    return np.ascontiguousarray(np.asarray(x, dtype=np.float32))


# ---------------------------------------------------------------------------
# blend helpers: up16 along last dim (48 -> 96) / middle dim
# ---------------------------------------------------------------------------

def _up_last(nc, eng, out, xin, eng2=None):
    """xin [P, n, 48] -> out [P, n, 96], out = 16 * bilinear (x4 per axis)."""
    eng2 = eng2 or eng
    eng.tensor_scalar_mul(out[:, :, 0:1], xin[:, :, 0:1], 4.0)
    eng.tensor_scalar_mul(out[:, :, 95:96], xin[:, :, 47:48], 4.0)
    eng.scalar_tensor_tensor(out[:, :, 2:95:2], xin[:, :, 1:48], 3.0, xin[:, :, 0:47],
                             AL.mult, AL.add)
    eng2.scalar_tensor_tensor(out[:, :, 1:94:2], xin[:, :, 0:47], 3.0, xin[:, :, 1:48],
                              AL.mult, AL.add)


def _up_mid(nc, eng, out, xin, eng2=None):
    """xin [P, 48, n] -> out [P, 96, n]."""
    eng2 = eng2 or eng
    eng.tensor_scalar_mul(out[:, 0:1, :], xin[:, 0:1, :], 4.0)
    eng.tensor_scalar_mul(out[:, 95:96, :], xin[:, 47:48, :], 4.0)
    eng.scalar_tensor_tensor(out[:, 2:95:2, :], xin[:, 1:48, :], 3.0, xin[:, 0:47, :],
                             AL.mult, AL.add)
    eng2.scalar_tensor_tensor(out[:, 1:94:2, :], xin[:, 0:47, :], 3.0, xin[:, 1:48, :],
                              AL.mult, AL.add)


# ---------------------------------------------------------------------------
# device kernel
# ---------------------------------------------------------------------------

def build(debug_taps=False):
    nc = bacc.Bacc("TRN2", target_bir_lowering=False, debug=False, num_devices=N_CORES)

    low_d = nc.dram_tensor("low", [C, H, W], F32, kind="ExternalInput")
    high_d = nc.dram_tensor("high", [C, HS, HS], F32, kind="ExternalInput")
    wq_d = nc.dram_tensor("wqT", [C, 128], BF16, kind="ExternalInput")
    wkvx_d = nc.dram_tensor("wkvxT", [C, CIN + C + 2 * CI], BF16, kind="ExternalInput")
    wk2_d = nc.dram_tensor("wk2T", [CIN, 2 * CI], BF16, kind="ExternalInput")
    wv2_d = nc.dram_tensor("wv2T", [CIN, CIN], BF16, kind="ExternalInput")
    wfin_d = nc.dram_tensor("wfinT", [CIN, C], BF16, kind="ExternalInput")
    bnsc_d = nc.dram_tensor("bnsc", [C], F32, kind="ExternalInput")
    bnbi_d = nc.dram_tensor("bnbi", [C], F32, kind="ExternalInput")
    out_d = nc.dram_tensor("out", [C, H, W], F32, kind="ExternalOutput")

    taps = {}
    if debug_taps:
        for nm, shp in [("q", [CI, H, W]), ("k1", [CI, H, W]), ("v1", [CIN, H, W]),
                        ("expH", [96, W, 96]), ("A1", [CIN, H, W]), ("y", [C, H, W]),
                        ("k2", [CI, H, W])]:
            taps[nm] = nc.dram_tensor("t_" + nm, shp, F32, kind="ExternalOutput")

    ident = nc.inline_tensor(np.eye(128, dtype=ml_dtypes.bfloat16), "ident")
    ones_l = nc.inline_tensor(np.ones((96, 128), dtype=ml_dtypes.bfloat16), "ones_l")
    epsv = nc.inline_tensor(np.full((128, 1), BN_EPS, np.float32), "epsv")
    dmsk = np.ones((96, 4, 96), dtype=ml_dtypes.bfloat16)
    for _p in range(96):
        dmsk[_p, :, _p] = 0
    dmsk_c = nc.inline_tensor(dmsk.reshape(96, 384), "dmskc")

    st_d = [nc.dram_tensor(f"st{g}_i", [128, 2], F32) for g in range(4)]
    stg_d = [nc.dram_tensor(f"stg{g}_i", [128, 2], F32, addr_space="Shared") for g in range(4)]

    NKV = CIN + C + 2 * CI  # 832: [vs 0:256 | xs 256:768 | ks(x2) 768:832]

    with tile.TileContext(nc) as tc, (
        tc.tile_pool(name="cst", bufs=1)) as cst, (
        tc.tile_pool(name="per", bufs=1)) as per, (
        tc.tile_pool(name="strm", bufs=3)) as strm, (
        tc.tile_pool(name="pe", bufs=3, space="PSUM")) as pe, (
        tc.tile_pool(name="ps", bufs=1, space="PSUM")) as ps, (
        tc.tile_pool(name="pa", bufs=1, space="PSUM")) as pa, (
        tc.tile_pool(name="pb", bufs=2, space="PSUM")) as pb:

        # ---------------- consts & weights ----------------
        id_t = cst.tile([128, 128], BF16, tag="id")
        nc.sync.dma_start(id_t[:], ident.ap()[:])
        ones_t = cst.tile([96, 128], BF16, tag="ones")
        nc.sync.dma_start(ones_t[:], ones_l.ap()[:])
        eps_t = cst.tile([128, 1], F32, tag="eps")
        nc.sync.dma_start(eps_t[:], epsv.ap()[:])
        dmsk_t = cst.tile([96, 4, 96], BF16, tag="dmsk")
        nc.sync.dma_start(dmsk_t[:].rearrange("p a b -> p (a b)"), dmsk_c.ap()[:])

        wq_t = [cst.tile([128, 128], BF16, tag=f"wq{k}", name=f"wq{k}") for k in range(4)]
        for k in range(4):
            nc.sync.dma_start(wq_t[k][:], wq_d.ap()[k * 128:(k + 1) * 128, :])
        wk2_t = [cst.tile([128, 2 * CI], BF16, tag=f"wk2{k}", name=f"wk2{k}") for k in range(2)]
        wv2_t = [[cst.tile([128, 128], BF16, tag=f"wv2{k}{m}", name=f"wv2{k}{m}") for m in range(2)] for k in range(2)]
        wfin_t = [[cst.tile([128, 128], BF16, tag=f"wf{k}{m}", name=f"wf{k}{m}") for m in range(4)] for k in range(2)]
        for k in range(2):
            nc.scalar.dma_start(wk2_t[k][:], wk2_d.ap()[k * 128:(k + 1) * 128, :])
            for m in range(2):
                nc.scalar.dma_start(wv2_t[k][m][:], wv2_d.ap()[k * 128:(k + 1) * 128, m * 128:(m + 1) * 128])
            for m in range(4):
                nc.scalar.dma_start(wfin_t[k][m][:], wfin_d.ap()[k * 128:(k + 1) * 128, m * 128:(m + 1) * 128])
        bnsc_t = cst.tile([128, 4], F32, tag="bnsc")
        bnbi_t = cst.tile([128, 4], F32, tag="bnbi")
        nc.sync.dma_start(bnsc_t[:], bnsc_d.ap().rearrange("(m p) -> p m", p=128))
        nc.sync.dma_start(bnbi_t[:], bnbi_d.ap().rearrange("(m p) -> p m", p=128))

        # ---------------- persistent: A accumulation, xs, stats ----------------
        A1 = [per.tile([128, H, W], BF16, tag=f"A1{i}", name=f"A1{i}") for i in range(2)]  # (c, h, w)
        xs_t = [per.tile([128, HS, HS], BF16, tag=f"xs{i}", name=f"xs{i}") for i in range(4)]
        s1p = per.tile([128, 4, 24], F32, tag="s1p")
        s2p = per.tile([128, 4, 24], F32, tag="s2p")

        # ---------------- attention scratch (closes before y phase) ----------
        atta_cm = tc.tile_pool(name="atta", bufs=1)
        atta = atta_cm.__enter__()
        qr_t = atta.tile([128, H, W], BF16, tag="qr_t")   # q' @0:32 and @64:96
        kk_t = atta.tile([64, H, W], BF16, tag="kk_t")    # k' x2 quadrants; k2 overwrites k1 in place
        v1 = [atta.tile([128, H, W], BF16, tag=f"v1{i}", name=f"v1{i}") for i in range(2)]  # (c, h, w); becomes v2 in place

        # ---------------- Phase 1+3: high stream on small grid, upsample -------
        with tc.tile_pool(name="ph13", bufs=1) as ph:
            wkvx_t = [[ph.tile([128, 128], BF16, tag=f"wkvx{m}_{k}", name=f"wkvx{m}_{k}") for k in range(4)] for m in range(7)]
            wkvx_q = [nc.sync, nc.scalar]
            for m in range(7):
                mw = min(128, NKV - m * 128)
                for k in range(4):
                    wkvx_q[(m * 4 + k) % 2].dma_start(wkvx_t[m][k][:, 0:mw],
                                        wkvx_d.ap()[k * 128:(k + 1) * 128, m * 128:m * 128 + mw])

            vs_t = [ph.tile([128, HS, HS], BF16, tag=f"vs{i}", name=f"vs{i}") for i in range(2)]
            ks_t = ph.tile([2 * CI, HS, HS], BF16, tag="ks")

            for n0 in range(0, PIXS, 512):
                nn = min(512, PIXS - n0)
                hi_c = [strm.tile([128, 512], BF16, tag=f"hic{k}", name=f"hic{k}", bufs=2) for k in range(4)]
                for k in range(4):
                    nc.gpsimd.dma_start(hi_c[k][:, 0:nn],
                                        high_d.ap().rearrange("c a b -> c (a b)")[k * 128:(k + 1) * 128, n0:n0 + nn])
                for m in range(7):
                    mw = min(128, NKV - m * 128)
                    pm = pb.tile([128, 512], F32, tag="pmm")
                    for k in range(4):
                        nc.tensor.matmul(pm[0:mw, 0:nn], wkvx_t[m][k][:, 0:mw],
                                         hi_c[k][:, 0:nn], start=(k == 0), stop=(k == 3))
                    if m < 2:
                        dst = vs_t[m][:].rearrange("c a b -> c (a b)")[:, n0:n0 + nn]
                    elif m < 6:
                        dst = xs_t[m - 2][:].rearrange("c a b -> c (a b)")[:, n0:n0 + nn]
                    else:
                        dst = ks_t[:].rearrange("c a b -> c (a b)")[:, n0:n0 + nn]
                    nc.scalar.activation(dst, pm[0:mw, 0:nn], AF.Copy)

            # upsample k1 (into qk[32:64]) and v1
            kw_t = ph.tile([2 * CI, HS, W], BF16, tag="kw")
            _up_last(nc, nc.vector, kw_t[:], ks_t[:])
            _up_mid(nc, nc.vector, kk_t[0:64], kw_t[:])
            for ct in range(2):
                vw_t = ph.tile([128, HS, W], BF16, tag="vw", name="vw", bufs=1)
                _up_last(nc, nc.vector, vw_t[:], vs_t[ct][:])
                _up_mid(nc, nc.vector, v1[ct][:], vw_t[:])

        # ---------------- round-only scratch ----------------
        attb_cm = tc.tile_pool(name="attb", bufs=1)
        attb = attb_cm.__enter__()
        expH = attb.tile([96, W, 96], BF16, tag="expH")   # [H', w, h]
        expW = attb.tile([96, H, 96], BF16, tag="expW")   # [W', h, w]

        # ---------------- Phase 2: q from low ----------------
        for n0 in range(0, PIX, 512):
            low_c = [strm.tile([128, 512], BF16, tag=f"hic{k}", name=f"lowc{k}", bufs=2) for k in range(4)]
            for k in range(4):
                nc.gpsimd.dma_start(low_c[k][:],
                                    low_d.ap().rearrange("c a b -> c (a b)")[k * 128:(k + 1) * 128, n0:n0 + 512])
            pm = pb.tile([128, 512], F32, tag="pmm", name="pmq")
            for k in range(4):
                nc.tensor.matmul(pm[:], wq_t[k][:], low_c[k][:], start=(k == 0), stop=(k == 3))
            nc.scalar.activation(qr_t[:].rearrange("c a b -> c (a b)")[:, n0:n0 + 512], pm[:], AF.Copy)

        # ---------------- attention helpers ----------------
        def energies(kbase):
            for w0 in range(0, W, 4):
                pes = pe.tile([96, 4, 96], F32, tag="pe")
                for j in range(4):
                    w = w0 + j
                    nc.tensor.matmul(pes[:, j, :], kk_t[0:32, :, w], qr_t[0:32, :, w],
                                     start=True, stop=True)
                nc.scalar.activation(expH[:, w0:w0 + 4, :], pes[:], AF.Exp)
                nc.gpsimd.tensor_mul(expH[:, w0:w0 + 4, :], expH[:, w0:w0 + 4, :], dmsk_t[:])
            for h0 in range(0, H, 4):
                pes = pe.tile([96, 4, 96], F32, tag="pe")
                for j in range(4):
                    h = h0 + j
                    nc.tensor.matmul(pes[:, j, :], kk_t[0:32, h, :], qr_t[0:32, h, :],
                                     start=True, stop=True)
                nc.scalar.activation(expW[:, h0:h0 + 4, :], pes[:], AF.Exp)

        def softmax_norm():
            expWv = expW[:].rearrange("p h w -> p w h")
            for w0 in range(0, W, 4):
                pss = ps.tile([128, 4, 96], F32, tag="ps")
                nc.tensor.matmul(pss[:], ones_t[:], expH[:, w0:w0 + 4, :], start=True, stop=False)
                nc.tensor.matmul(pss[:], ones_t[:], expWv[:, w0:w0 + 4, :], start=False, stop=True)
                srec = strm.tile([128, 4, 96], F32, tag="srec")
                nc.vector.reciprocal_approx_fast(srec[:], pss[:])
                nc.vector.tensor_mul(expH[:, w0:w0 + 4, :], expH[:, w0:w0 + 4, :], srec[0:96])
                nc.gpsimd.tensor_mul(expWv[:, w0:w0 + 4, :], expWv[:, w0:w0 + 4, :], srec[0:96])

        def aggregate(rnd, v):
            # W direction first: per-row h, natural (c, h, w) dst
            for h0 in range(0, H, 4):
                vtc = strm.tile([96, 4, 256], BF16, tag="vtc")
                for j in range(4):
                    h = h0 + j
                    for ct in range(2):
                        pt = pe.tile([96, 128], BF16, tag="pe")
                        nc.tensor.transpose(pt[:], v[ct][:, h, :], id_t[:])
                        nc.scalar.activation(vtc[:, j, ct * 128:(ct + 1) * 128], pt[:], AF.Copy)
                for half in range(2):
                    pag = pa.tile([128, 4, 96], F32, tag=f"pa{half}")
                    for j in range(4):
                        nc.tensor.matmul(pag[:, j, :], vtc[:, j, half * 128:(half + 1) * 128],
                                         expW[:, h0 + j, :], start=True, stop=True)
                    if rnd == 0:
                        nc.scalar.activation(A1[half][:, h0:h0 + 4, :], pag[:], AF.Copy)
                    else:
                        nc.vector.scalar_tensor_tensor(A1[half][:, h0:h0 + 4, :], pag[:], 1.0,
                                                       A1[half][:, h0:h0 + 4, :], AL.mult, AL.add)
            # H direction: per-column w, strided (c, w, h) view dst
            for w0 in range(0, W, 4):
                vtc = strm.tile([96, 4, 256], BF16, tag="vtc")
                for j in range(4):
                    w = w0 + j
                    for ct in range(2):
                        pt = pe.tile([96, 128], BF16, tag="pe")
                        nc.tensor.transpose(pt[:], v[ct][:, :, w], id_t[:])
                        nc.scalar.activation(vtc[:, j, ct * 128:(ct + 1) * 128], pt[:], AF.Copy)
                for half in range(2):
                    pag = pa.tile([128, 4, 96], F32, tag=f"pa{half}")
                    for j in range(4):
                        nc.tensor.matmul(pag[:, j, :], vtc[:, j, half * 128:(half + 1) * 128],
                                         expH[:, w0 + j, :], start=True, stop=True)
                    dst = A1[half][:].rearrange("c h w -> c w h")[:, w0:w0 + 4, :]
                    nc.vector.scalar_tensor_tensor(dst, pag[:], 1.0, dst, AL.mult, AL.add)

        # ---------------- round 1 ----------------
        energies(0)
        softmax_norm()
        aggregate(0, v1)

        # ---------------- round 2 prep (h-strips, all natural) ----------------
        for h0 in range(0, H, 4):
            pm = pb.tile([2 * CI, 4, 96], F32, tag="pmm")
            for k in range(2):
                nc.tensor.matmul(pm[:].rearrange("c a b -> c (a b)"), wk2_t[k][:],
                                 A1[k][:].rearrange("c h w -> c (h w)")[:, h0 * 96:(h0 + 4) * 96],
                                 start=(k == 0), stop=(k == 1))
            nc.vector.scalar_tensor_tensor(kk_t[0:64, h0:h0 + 4, :], pm[:], 1.0,
                                           kk_t[0:64, h0:h0 + 4, :], AL.mult, AL.add)
        for h0 in range(0, H, 4):
            for m in range(2):
                pm = pb.tile([128, 4, 96], F32, tag="pmm")
                for k in range(2):
                    nc.tensor.matmul(pm[:].rearrange("c a b -> c (a b)"), wv2_t[k][m][:],
                                     A1[k][:].rearrange("c h w -> c (h w)")[:, h0 * 96:(h0 + 4) * 96],
                                     start=(k == 0), stop=(k == 1))
                nc.vector.scalar_tensor_tensor(v1[m][:, h0:h0 + 4, :], pm[:], 1.0,
                                               v1[m][:, h0:h0 + 4, :], AL.mult, AL.add)

        # ---------------- round 2 ----------------
        energies(0)
        softmax_norm()
        aggregate(1, v1)

        if taps:
            nc.gpsimd.dma_start(taps["q"].ap().rearrange("c a b -> c (a b)"),
                              qr_t[0:32].rearrange("c a b -> c (a b)"))
            nc.gpsimd.dma_start(taps["k1"].ap().rearrange("c a b -> c (a b)"),
                              kk_t[0:32].rearrange("c a b -> c (a b)"))
            nc.gpsimd.dma_start(taps["k2"].ap().rearrange("c a b -> c (a b)"),
                              kk_t[0:32].rearrange("c a b -> c (a b)"))
            for ct in range(2):
                nc.gpsimd.dma_start(taps["v1"].ap().rearrange("c a b -> c (a b)")[ct * 128:(ct + 1) * 128, :],
                                  v1[ct][:].rearrange("c a b -> c (a b)"))
            nc.gpsimd.dma_start(taps["expH"].ap().rearrange("c a b -> c (a b)"),
                              expH[:].rearrange("c a b -> c (a b)"))

        attb_cm.__exit__(None, None, None)
        atta_cm.__exit__(None, None, None)

        # ---------------- final y (SBUF-resident) + stats ----------------
        yp_cm = tc.tile_pool(name="yp", bufs=1)
        yp = yp_cm.__enter__()
        ybig = [yp.tile([128, H, W], BF16, tag=f"yb{m}", name=f"yb{m}") for m in range(4)]

        def final_m(m, weave=None):
            xw_t = yp.tile([128, HS, W], BF16, tag="xw", name="xw", bufs=1)
            _up_last(nc, nc.vector, xw_t[:], xs_t[m][:])
            x1m = yp.tile([128, H, W], BF16, tag="x1m", name="x1m", bufs=1)
            _up_mid(nc, nc.vector, x1m[:], xw_t[:])
            for hi, h0 in enumerate(range(0, H, 4)):
                pm = pa.tile([128, 4, 96], F32, tag=f"pa{hi % 2}")
                for k in range(2):
                    nc.tensor.matmul(pm[:].rearrange("c a b -> c (a b)"), wfin_t[k][m][:],
                                     A1[k][:].rearrange("c h w -> c (h w)")[:, h0 * 96:(h0 + 4) * 96],
                                     start=(k == 0), stop=(k == 1))
                nc.vector.scalar_tensor_tensor(ybig[m][:, h0:h0 + 4, :], pm[:], 1.0,
                                               x1m[:, h0:h0 + 4, :], AL.mult, AL.add,
                                               accum_out=s1p[:, m, hi].unsqueeze(1))
                junk = strm.tile([128, 4, 96], BF16, tag="junk")
                nc.scalar.activation(junk[:], ybig[m][:, h0:h0 + 4, :], AF.Square,
                                     accum_out=s2p[:, m, hi].unsqueeze(1))
                if weave and hi % 3 == 2:
                    weave.pop(0)()

        def stats_m(g):
            # stats AllReduce for m-tile g; returns (a, b) [128, 1]
            st_t = per.tile([128, 2], F32, tag=f"st{g}")
            nc.vector.tensor_reduce(st_t[:, 0:1], s1p[:, g:g + 1, :], mybir.AxisListType.X, AL.add)
            nc.vector.tensor_reduce(st_t[:, 1:2], s2p[:, g:g + 1, :], mybir.AxisListType.X, AL.add)
            nc.sync.dma_start(st_d[g].ap()[:], st_t[:])
            nc.gpsimd.collective_compute("AllReduce", AL.add, ins=[st_d[g].ap()], outs=[stg_d[g].ap()],
                                         replica_groups=[list(range(N_CORES))])
            stg_t = per.tile([128, 2], F32, tag=f"stg{g}")
            nc.sync.dma_start(stg_t[:], stg_d[g].ap()[:])
            mean_t = per.tile([128, 1], F32, tag=f"mean{g}")
            var_t = per.tile([128, 1], F32, tag=f"var{g}")
            nc.vector.tensor_scalar_mul(mean_t[:], stg_t[:, 0:1], 1.0 / NTOT)
            nc.vector.tensor_scalar_mul(var_t[:], stg_t[:, 1:2], 1.0 / NTOT)
            m2_t = per.tile([128, 1], F32, tag=f"m2{g}")
            nc.vector.tensor_mul(m2_t[:], mean_t[:], mean_t[:])
            nc.vector.tensor_sub(var_t[:], var_t[:], m2_t[:])
            sd_t = per.tile([128, 1], F32, tag=f"sd{g}")
            nc.scalar.activation(sd_t[:], var_t[:], AF.Sqrt, bias=eps_t[:, 0:1])
            ri_t = per.tile([128, 1], F32, tag=f"ri{g}")
            nc.vector.reciprocal(ri_t[:], sd_t[:])
            a_t = per.tile([128, 1], F32, tag=f"abn{g}")
            b_t = per.tile([128, 1], F32, tag=f"bbn{g}")
            nc.vector.tensor_mul(a_t[:], ri_t[:], bnsc_t[:, g:g + 1])
            nc.vector.tensor_mul(b_t[:], a_t[:], mean_t[:])
            nc.vector.tensor_sub(b_t[:], bnbi_t[:, g:g + 1], b_t[:])
            return a_t, b_t

        def apply_chunk(m, a_t, b_t, hi, h0):
            oc = strm.tile([128, 12, 96], F32, tag="obn", bufs=3)
            nc.scalar.activation(oc[:], ybig[m][:, h0:h0 + 12, :], AF.Relu,
                                 scale=a_t[:, 0:1], bias=b_t[:, 0:1])
            eng = [nc.sync, nc.gpsimd][(m * 8 + hi) % 2]
            eng.dma_start(out_d.ap()[m * 128:(m + 1) * 128, h0:h0 + 12, :], oc[:])

        def apply_chunks(m, a_t, b_t):
            return [
                (lambda hi=hi, h0=h0: apply_chunk(m, a_t, b_t, hi, h0))
                for hi, h0 in enumerate(range(0, H, 12))
            ]

        final_m(0)
        ab0 = stats_m(0)
        final_m(1)
        ab1 = stats_m(1)
        final_m(2, weave=apply_chunks(0, *ab0))   # m0 applies fill scalar gaps
        ab2 = stats_m(2)
        final_m(3, weave=apply_chunks(1, *ab1))
        ab3 = stats_m(3)
        for f in apply_chunks(2, *ab2) + apply_chunks(3, *ab3):
            f()

        # ---------------- debug taps ----------------
        if taps:
            for ct in range(2):
                nc.gpsimd.dma_start(taps["A1"].ap().rearrange("c a b -> c (a b)")[ct * 128:(ct + 1) * 128, :],
                                  A1[ct][:].rearrange("c a b -> c (a b)"))
            for m in range(4):
                nc.gpsimd.dma_start(taps["y"].ap().rearrange("c a b -> c (a b)")[m * 128:(m + 1) * 128, :],
                                  ybig[m][:].rearrange("c a b -> c (a b)"))

        yp_cm.__exit__(None, None, None)

    nc.compile()
    return nc


# ---------------------------------------------------------------------------
# host entry
# ---------------------------------------------------------------------------

def _host_prep(inputs):
    conv1_w = _f32(inputs["conv1_w"]); conv2_w = _f32(inputs["conv2_w"])
    q_w = _f32(inputs["q_w"]); k_w = _f32(inputs["k_w"]); v_w = _f32(inputs["v_w"])
    gamma = float(np.asarray(inputs["gamma"]))
    wb = _f32(inputs["bottleneck_w"])
    wb_v, wb_h = wb[:, :CIN], wb[:, CIN:]

    wq = (q_w @ conv1_w) / 16.0
    wvs = v_w @ conv2_w
    wxs = (wb_v @ conv2_w + wb_h) / 16.0
    wks = k_w @ conv2_w
    wkvx = np.concatenate([wvs, wxs, wks, wks], axis=0)

    def _bf16(x):
        return np.ascontiguousarray(np.asarray(x, dtype=ml_dtypes.bfloat16))

    return {
        "wqT": _bf16(np.tile(wq.T, (1, 4))),
        "wkvxT": _bf16(wkvx.T),
        "wk2T": _bf16(np.tile((gamma * k_w).T, (1, 2))),
        "wv2T": _bf16((gamma * v_w).T),
        "wfinT": _bf16((gamma / 16.0 * wb_v).T),
        "bnsc": _f32(inputs["bn_scale"]),
        "bnbi": _f32(inputs["bn_bias"]),
    }


def _get_nc(debug_taps=False):
    key = ("nc", debug_taps)
    if key not in _CACHE:
        _CACHE[key] = build(debug_taps)
    return _CACHE[key]


def run(inputs, debug_taps=False, trace=False):
    for bname in ("conv1_b", "conv2_b", "q_b", "k_b", "v_b"):
        assert np.abs(np.asarray(inputs[bname])).max() == 0.0, f"nonzero {bname} unsupported"
    shared = _host_prep(inputs)
    low = _f32(inputs["low_feature"])
    high = _f32(inputs["high_feature"])
    in_maps = [dict(shared, low=low[i], high=high[i]) for i in range(N_CORES)]
    nc = _get_nc(debug_taps)
    res = run_bass_kernel_spmd(nc, in_maps, core_ids=list(range(N_CORES)), trace=trace)
    return res


def kernel(**inputs):
    res = run(inputs)
    out = np.stack([res.results[i]["out"] for i in range(N_CORES)], axis=0)
    return out.astype(np.float32)
